# revision 29
# baseline (speedup 1.0000x reference)
"""Trainium2 Bass kernel for nn_Attention_46067819217077 (sparse_attention).

Computation (reference):
  x (64,2,32,32) -> flatten (b=64, n=2, dim=1024)
  q/k/v = BN1d_n( x @ W{q,k,v}.T )          (inner = 2048 = 2 heads x 1024)
  linear attention per (b, head):  out = (s @ v_hat) * D_inv  with
      s[n,m] = q_hat[n] . k_hat[m],  D[n] = s[n,0]+s[n,1]   (seq len n = 2)
  out = merge_heads @ Wo.T + bo ; BN2d over (b, H, W)

Strategy: 8-way tensor-parallel on the inner (head-feature) axis.
Each core owns a 256-wide chunk of Wq/Wk/Wv rows and of Wo columns.
BatchNorm statistics, q.k dot products and row sums are all *linear* in
per-chunk partial sums, so small collectives provide everything needed.

v1 (legacy): AllGather tiny payload -> local sum -> compute attention
  scalars -> combine into per-core partial -> AllReduce [128,1024] f32.
  The two collectives are serially dependent.

v2: two *independent* collectives. CC-A: AllReduce the [128,16] f32
  payload. CC-B: AllReduce raw Y = V @ WoC.T, head-packed into
  [128,2048] fp16 (head-0 cores fill cols 0:1024, head-1 cores cols
  1024:2048; zeros elsewhere). All attention/BN scalar math runs while
  CC-B is in flight; the final combine applies per-row scalars to the
  two head sums:
    X = ud0*Y0 + uo0*Y0_sw + ud1*Y1 + uo1*Y1_sw + z0*wos0 + z1*wos1 + bo
  BN2 is computed redundantly on every core. V/Wo matmuls run in fp16
  (the q/k path must stay fp32: 1/D amplifies q/k rounding ~20x).

v2m: like v2 but one merged f32 AllReduce [128, 2064] (Y head-packed
  f32 + payload in the last 16 cols).

v3/v3m: v2/v2m + cross-rep pipelining (bufs=2 pools, parity-double-
  buffered collective staging tensors).

v6: v3 + software-pipelined emission: phase1(k+1) is emitted before
  phase2(k) so no engine queue blocks the next rep's collective inputs
  behind the current rep's post-collective tail.

v7 (default, fastest): v6 with ONE int16 AllReduce [128, 2080] per rep.
  Measured: each collective ring op costs ~9us latency regardless of
  size and consecutive collectives never overlap, so collective COUNT
  dominates. Y rides as int16 (scale sY derived from the actual inputs
  host-side, with 2x margins); the payload needs f32-grade precision
  (1/D amplifies errors ~20x) which fp16 wire-adds cannot give, but
  integer wire-adds are EXACT: each f32 payload value v is carried as
  hi = i16(v*s/2048), lo = i16(v*s - 2048*hi) and decoded as
  (sum(hi)*2048 + sum(lo))/s after the reduce. HW rel err 4.4e-4.

Row layout everywhere: r = n*64 + b  (channel-major, 128 rows).
"""

import numpy as np

NC = 8
B, N, H, W = 64, 2, 32, 32
DIM = H * W                # 1024
INNER = DIM * 2            # 2048
DPC = INNER // NC          # 256 per-core chunk
EPS = 1e-5

_PROG_CACHE = {}

# Matmul operand dtype for the q/k path: "f32" (exact, 4 cycles/row) or
# "f32r" (fast fp32, reduced precision). V/Wo path in v2 is always fp16.
MM_DT = "f32r"    # QK-path matmul dtype; f32r measured 5.9e-3 rel err on HW
STRATEGY = "v9"   # v9 = v8 + head-pack on act engine (see _build_v6 act_pack)
NO_CC = False     # False | True | "no_a" | "no_b"  (local-DMA substitutes)
CC_F32 = False    # CC-B (Y AllReduce) in f32 instead of fp16
CC_DIM = "Free"   # cc_dim for the packed AllReduce ("Free" benched ~5-7us faster)
CC_UT = "No"      # unique_tensors hint for the packed AllReduce


def _build_program(mm_dt=None, reps=1, strategy=None):
    strategy = strategy or STRATEGY
    if strategy == "v1":
        return _build_v1(mm_dt, reps)
    if strategy == "v6":
        return _build_v6(mm_dt, reps)
    if strategy == "v7":
        return _build_v6(mm_dt, reps, packed=True)
    if strategy == "v8":
        return _build_v6(mm_dt, reps, packed=True, slim=True)
    if strategy == "v9":
        return _build_v6(mm_dt, reps, packed=True, slim=True, act_pack=True)
    return _build_v2(mm_dt, reps, merged=(strategy in ("v2m", "v3m")),
                     pipelined=(strategy in ("v3", "v3m")))


# --------------------------------------------------------------------------
# v2: independent collectives, fp16 V/Wo path
# --------------------------------------------------------------------------

def _build_v2(mm_dt=None, reps=1, merged=False, pipelined=False):
    import concourse.bass as bass
    import concourse.mybir as mybir
    import concourse.tile as tile
    from concourse import bacc

    f32 = mybir.dt.float32
    f16 = mybir.dt.float16
    fmm = mybir.dt.float32r if (mm_dt or MM_DT) == "f32r" else mybir.dt.float32
    fcc = f32 if (merged or CC_F32) else f16   # dtype of the big Y collective
    A = mybir.AluOpType
    AF = mybir.ActivationFunctionType
    AX = mybir.AxisListType

    no_cc = NO_CC
    nc = bacc.Bacc(None, target_bir_lowering=False, debug=False, num_devices=NC)

    # ---- I/O ----
    d_xt = nc.dram_tensor("xt", [128, 8, 128], fmm, kind="ExternalInput")
    d_wqk = nc.dram_tensor("wqk", [128, 8, 512], fmm, kind="ExternalInput")
    d_wv = nc.dram_tensor("wv", [128, 8, 256], f16, kind="ExternalInput")
    d_wo = nc.dram_tensor("wo", [128, 2, 1024], f16, kind="ExternalInput")
    d_wos = nc.dram_tensor("wos", [2, 1024], f32, kind="ExternalInput")
    d_bo = nc.dram_tensor("bo", [1024], f32, kind="ExternalInput")
    d_hm = nc.dram_tensor("hm", [128, 2], f32, kind="ExternalInput")
    d_mn = nc.dram_tensor("mn", [128, 2], f32, kind="ExternalInput")
    d_m2 = nc.dram_tensor("m2", [2, 128], f32, kind="ExternalInput")
    d_gb = nc.dram_tensor("gb", [2, 8], f32, kind="ExternalInput")
    d_out = nc.dram_tensor("out", [128, 1024], f32, kind="ExternalOutput")

    ncc = 2 if pipelined else 1   # double-buffer collective staging by rep parity
    if merged:
        ccm_in = [nc.dram_tensor(f"ccm_in{i}", [128, 2064], f32) for i in range(ncc)]
        ccm_out = [nc.dram_tensor(f"ccm_out{i}", [128, 2064], f32, addr_space="Shared")
                   for i in range(ncc)]
    else:
        cc1_in = [nc.dram_tensor(f"cc1_in{i}", [128, 16], f32) for i in range(ncc)]
        cc1_out = [nc.dram_tensor(f"cc1_out{i}", [128, 16], f32, addr_space="Shared")
                   for i in range(ncc)]
        cc2_in = [nc.dram_tensor(f"cc2_in{i}", [128, 2048], fcc) for i in range(ncc)]
        cc2_out = [nc.dram_tensor(f"cc2_out{i}", [128, 2048], fcc, addr_space="Shared")
                   for i in range(ncc)]

    def bcast(ap, p=128):
        return bass.AP(tensor=ap.tensor, offset=ap.offset, ap=[[0, p]] + list(ap.ap))

    groups = [list(range(NC))]

    cb = 2 if pipelined else 1
    with tile.TileContext(nc) as tc:
        with (
            tc.tile_pool(name="const", bufs=cb) as cst_pool,
            tc.tile_pool(name="work", bufs=1) as wk,
            tc.tile_pool(name="psum", bufs=1, space="PSUM") as ps,
            tc.tile_pool(name="psvt", bufs=2, space="PSUM") as psvt,
        ):
            for _rep in range(reps):
                pp = _rep % ncc
                # ---- constant loads (chunked for DMA/compute overlap) ----
                t_xt = [cst_pool.tile([128, 128], fmm, name=f"xt{c}", tag=f"xt{c}") for c in range(8)]
                t_wqk = [cst_pool.tile([128, 512], fmm, name=f"wqk{c}", tag=f"wqk{c}") for c in range(8)]
                t_wv = [cst_pool.tile([128, 256], f16, name=f"wv{c}", tag=f"wv{c}") for c in range(8)]
                t_wo = [cst_pool.tile([128, 1024], f16, name=f"wo{k}", tag=f"wo{k}") for k in range(2)]
                t_wos = cst_pool.tile([128, 2, 1024], f32, name="wos", tag="wos")
                t_bo = cst_pool.tile([128, 1024], f32, name="bo", tag="bo")
                t_hm = cst_pool.tile([128, 2], f32, name="hm", tag="hm")
                t_mn = cst_pool.tile([128, 2], f32, name="mn", tag="mn")
                t_m2 = cst_pool.tile([2, 128], f32, name="m2", tag="m2")
                t_gb = cst_pool.tile([2, 8], f32, name="gb", tag="gb")

                for c in range(8):
                    nc.sync.dma_start(out=t_xt[c], in_=d_xt[:, c, :])
                    nc.sync.dma_start(out=t_wqk[c], in_=d_wqk[:, c, :])
                for c in range(8):
                    nc.sync.dma_start(out=t_wv[c], in_=d_wv[:, c, :])
                for k in range(2):
                    nc.sync.dma_start(out=t_wo[k], in_=d_wo[:, k, :])
                nc.sync.dma_start(out=t_hm, in_=d_hm[:])
                nc.sync.dma_start(out=t_mn, in_=d_mn[:])
                nc.sync.dma_start(out=t_m2, in_=d_m2[:])
                nc.sync.dma_start(out=t_gb, in_=d_gb[:])
                nc.sync.dma_start(out=t_wos, in_=bcast(d_wos[:]))
                nc.sync.dma_start(out=t_bo, in_=bcast(d_bo[:]))

                # fp16 cast of x chunks for the V path
                t_x16 = [wk.tile([128, 128], f16, name=f"x16_{c}", tag=f"x16_{c}", bufs=cb)
                         for c in range(8)]
                for c in range(8):
                    nc.scalar.copy(out=t_x16[c], in_=t_xt[c])

                # ---- phase 1: projections ----
                qk_ps = ps.tile([128, 512], f32, name="qk", tag="qk", bufs=cb)
                for c in range(8):
                    nc.tensor.matmul(qk_ps, lhsT=t_xt[c], rhs=t_wqk[c],
                                     start=(c == 0), stop=(c == 7))
                # V^T tiles: out[j, r] += wv_c.T @ x16_c   (fp16)
                vt_ps = [psvt.tile([128, 128], f32, name="vt", tag="vt") for _ in range(2)]
                for half in range(2):
                    for c in range(8):
                        nc.tensor.matmul(vt_ps[half], lhsT=t_wv[c][:, half * 128:(half + 1) * 128],
                                         rhs=t_x16[c], start=(c == 0), stop=(c == 7))

                # ---- phase 2: payload (per-chunk partial sums) ----
                q_ap = qk_ps[:, 0:256]
                k_ap = qk_ps[:, 256:512]
                ksb = wk.tile([128, 256], f32, name="ksb", tag="ksb")
                nc.scalar.copy(out=ksb, in_=k_ap)
                ksw = wk.tile([128, 256], f32, name="ksw", tag="ksw")
                nc.vector.tensor_copy(out=ksw[0:64, :], in_=ksb[64:128, :])
                nc.vector.tensor_copy(out=ksw[64:128, :], in_=ksb[0:64, :])

                tmp4 = wk.tile([128, 4], f32, name="tmp4", tag="tmp4")
                prod1 = wk.tile([128, 256], f32, name="prod1", tag="prod1")
                prod2 = wk.tile([128, 256], f32, name="prod2", tag="prod2")
                nc.vector.tensor_tensor(out=prod1, in0=q_ap, in1=ksb, op=A.mult)
                nc.vector.tensor_reduce(out=tmp4[:, 0:1], in_=prod1, axis=AX.X, op=A.add)
                nc.vector.tensor_tensor(out=prod2, in0=q_ap, in1=ksw, op=A.mult)
                nc.vector.tensor_reduce(out=tmp4[:, 1:2], in_=prod2, axis=AX.X, op=A.add)
                nc.vector.tensor_reduce(out=tmp4[:, 2:4],
                                        in_=qk_ps[:].rearrange("p (t j) -> p t j", t=2),
                                        axis=AX.X, op=A.add)

                pay = wk.tile([128, 16], f32, name="pay", tag="pay", bufs=cb)
                nc.vector.memset(pay, 0.0)
                nc.vector.tensor_scalar(out=pay[:, 0:4], in0=tmp4, scalar1=t_hm[:, 0:1],
                                        scalar2=None, op0=A.mult)
                nc.vector.tensor_scalar(out=pay[:, 4:8], in0=tmp4, scalar1=t_hm[:, 1:2],
                                        scalar2=None, op0=A.mult)
                sq1 = wk.tile([128, 256], f32, name="sq1", tag="sq1")
                sq2 = wk.tile([128, 256], f32, name="sq2", tag="sq2")
                nc.scalar.activation(out=sq1, in_=q_ap, func=AF.Square, accum_out=pay[:, 8:9])
                nc.scalar.activation(out=sq2, in_=k_ap, func=AF.Square, accum_out=pay[:, 9:10])
                vsq = [wk.tile([128, 128], f32, name=f"vsq{i}", tag=f"vsq{i}") for i in range(2)]
                v2ab = [wk.tile([128, 2], f32, name=f"v2ab{i}", tag=f"v2ab{i}") for i in range(2)]
                for half in range(2):
                    nc.scalar.activation(out=vsq[half], in_=vt_ps[half], func=AF.Square)
                    nc.vector.tensor_reduce(out=v2ab[half],
                                            in_=vsq[half][:].rearrange("p (t r) -> p t r", t=2),
                                            axis=AX.X, op=A.add)
                nc.vector.tensor_tensor(out=pay[:, 11:13], in0=v2ab[0], in1=v2ab[1], op=A.add)
                vsab = [wk.tile([128, 2], f32, name=f"vsab{i}", tag=f"vsab{i}") for i in range(2)]
                for half in range(2):
                    nc.vector.tensor_reduce(out=vsab[half],
                                            in_=vt_ps[half][:].rearrange("p (t r) -> p t r", t=2),
                                            axis=AX.X, op=A.add)
                nc.vector.tensor_tensor(out=pay[:, 13:15], in0=vsab[0], in1=vsab[1], op=A.add)

                # ---- Y matmuls: Y = V @ WoC.T  (fp16) ----
                vts = wk.tile([128, 2, 128], f16, name="vts", tag="vts", bufs=cb)
                for half in range(2):
                    nc.scalar.copy(out=vts[:, half, :], in_=vt_ps[half])
                y_ps = ps.tile([128, 1024], f32, name="y", tag="y")
                for k in range(2):
                    for nn in range(2):
                        nc.tensor.matmul(y_ps[:, nn * 512:(nn + 1) * 512],
                                         lhsT=vts[:, k, :], rhs=t_wo[k][:, nn * 512:(nn + 1) * 512],
                                         start=(k == 0), stop=(k == 1))
                # head-pack Y into [128, 2048] via the per-core head mask
                yh = wk.tile([128, 2048], fcc, name="yh", tag="yh", bufs=cb)
                nc.vector.tensor_scalar(out=yh[:, 0:1024], in0=y_ps, scalar1=t_hm[:, 0:1],
                                        scalar2=None, op0=A.mult)
                nc.vector.tensor_scalar(out=yh[:, 1024:2048], in0=y_ps, scalar1=t_hm[:, 1:2],
                                        scalar2=None, op0=A.mult)

                # ---- collectives ----
                if merged:
                    nc.sync.dma_start(out=ccm_in[pp][:, 0:2048], in_=yh)
                    nc.sync.dma_start(out=ccm_in[pp][:, 2048:2064], in_=pay)
                    if no_cc:
                        nc.gpsimd.dma_start(out=ccm_out[pp][:], in_=ccm_in[pp][:])
                    else:
                        nc.gpsimd.collective_compute(
                            "AllReduce", A.add, replica_groups=groups,
                            ins=[ccm_in[pp][:]], outs=[ccm_out[pp][:]])
                    S = wk.tile([128, 16], f32, name="S", tag="S")
                    nc.sync.dma_start(out=S, in_=ccm_out[pp][:, 2048:2064])
                else:
                    # CC-A: tiny payload AllReduce (issued first, small)
                    nc.sync.dma_start(out=cc1_in[pp][:], in_=pay)
                    if no_cc in (True, "no_a"):
                        nc.gpsimd.dma_start(out=cc1_out[pp][:], in_=cc1_in[pp][:])
                    else:
                        nc.gpsimd.collective_compute(
                            "AllReduce", A.add, replica_groups=groups,
                            ins=[cc1_in[pp][:]], outs=[cc1_out[pp][:]])
                    # CC-B: head-packed Y AllReduce (independent of CC-A)
                    nc.sync.dma_start(out=cc2_in[pp][:], in_=yh)
                    if no_cc in (True, "no_b"):
                        nc.gpsimd.dma_start(out=cc2_out[pp][:], in_=cc2_in[pp][:])
                    else:
                        nc.gpsimd.collective_compute(
                            "AllReduce", A.add, replica_groups=groups,
                            ins=[cc2_in[pp][:]], outs=[cc2_out[pp][:]])
                    S = wk.tile([128, 16], f32, name="S", tag="S")
                    nc.sync.dma_start(out=S, in_=cc1_out[pp][:])

                # ---- global BN1 stats (overlaps CC-B flight) ----
                rhs4 = wk.tile([128, 4], f32, name="rhs4", tag="rhs4")
                nc.vector.tensor_tensor(out=rhs4[:, 0:2], in0=S[:, 2:4], in1=S[:, 6:8], op=A.add)
                nc.vector.tensor_copy(out=rhs4[:, 2:4], in_=S[:, 8:10])
                ones1 = wk.tile([128, 1], f32, name="ones1", tag="ones1")
                nc.vector.memset(ones1, 1.0)
                st_ps = ps.tile([128, 16], f32, name="st", tag="sm")
                nc.tensor.matmul(out=st_ps[0:2, 0:4], lhsT=t_mn, rhs=rhs4, start=True, stop=True)
                nc.tensor.matmul(out=st_ps[0:2, 4:5], lhsT=S[:, 11:13], rhs=ones1,
                                 start=True, stop=True)
                nc.tensor.matmul(out=st_ps[0:2, 5:6], lhsT=S[:, 13:15], rhs=ones1,
                                 start=True, stop=True)
                sts = wk.tile([2, 6], f32, name="sts", tag="sts")   # [Sq Sk Sv SSq SSk SSv]
                nc.vector.tensor_copy(out=sts[:, 0:2], in_=st_ps[0:2, 0:2])
                nc.vector.tensor_copy(out=sts[:, 2:3], in_=st_ps[0:2, 5:6])
                nc.vector.tensor_copy(out=sts[:, 3:5], in_=st_ps[0:2, 2:4])
                nc.vector.tensor_copy(out=sts[:, 5:6], in_=st_ps[0:2, 4:5])

                cst = wk.tile([2, 32], f32, name="cst", tag="cst")
                eps_t = wk.tile([2, 1], f32, name="eps_t", tag="eps_t")
                nc.vector.memset(eps_t, EPS)
                inv_n1 = 1.0 / float(B * INNER)
                nc.vector.tensor_scalar(out=cst[:, 0:3], in0=sts[:, 0:3], scalar1=inv_n1,
                                        scalar2=None, op0=A.mult)          # means
                nc.vector.tensor_scalar(out=cst[:, 3:6], in0=sts[:, 3:6], scalar1=inv_n1,
                                        scalar2=None, op0=A.mult)          # E[x^2]
                nc.vector.tensor_tensor(out=cst[:, 6:9], in0=cst[:, 0:3], in1=cst[:, 0:3], op=A.mult)
                nc.vector.tensor_tensor(out=cst[:, 9:12], in0=cst[:, 3:6], in1=cst[:, 6:9], op=A.subtract)
                nc.scalar.activation(out=cst[:, 12:15], in_=cst[:, 9:12], func=AF.Sqrt,
                                     bias=eps_t, scale=1.0)
                nc.vector.reciprocal(out=cst[:, 15:18], in_=cst[:, 12:15])
                nc.vector.tensor_tensor(out=cst[:, 18:21], in0=t_gb[0:2, 0:3], in1=cst[:, 15:18],
                                        op=A.mult)                          # A = g*rstd
                nc.vector.tensor_tensor(out=cst[:, 24:27], in0=cst[:, 18:21], in1=cst[:, 0:3],
                                        op=A.mult)                          # A*mean
                nc.vector.tensor_tensor(out=cst[:, 21:24], in0=t_gb[0:2, 3:6], in1=cst[:, 24:27],
                                        op=A.subtract)                      # C = b - A*mean

                # broadcast per-channel constants to rows: same + opposite channel
                bc_ps = ps.tile([128, 16], f32, name="bc", tag="vr")
                nc.tensor.matmul(out=bc_ps[:, 0:6], lhsT=t_m2, rhs=cst[:, 18:24],
                                 start=True, stop=True)
                bc = wk.tile([128, 12], f32, name="bc_sb", tag="bc_sb")
                nc.scalar.copy(out=bc[:, 0:6], in_=bc_ps[:, 0:6])
                # opposite-channel constants: swap row halves of bc[:,0:6]
                nc.vector.tensor_copy(out=bc[0:64, 6:12], in_=bc[64:128, 0:6])
                nc.vector.tensor_copy(out=bc[64:128, 6:12], in_=bc[0:64, 0:6])
                # bc cols: 0 Aq 1 Ak 2 Av 3 Cq 4 Ck 5 Cv | 6 Aq' 7 Ak' 8 Av' 9 Cq' 10 Ck' 11 Cv'

                # ---- scores: CACD coefficients (channel-based, head-independent) ----
                CACD = wk.tile([128, 8], f32, name="CACD", tag="CACD")
                in0 = bass.AP(tensor=bc.tensor, offset=bc.offset,
                              ap=[list(bc.ap[0]), [3, 2], [0, 4]])       # [Aq x4, Cq x4]
                in1 = bass.AP(tensor=bc.tensor, offset=bc.offset + 1,
                              ap=[list(bc.ap[0]), [0, 2], [3, 4]])       # [Ak Ck Ak' Ck'] x2
                nc.vector.tensor_tensor(out=CACD[:].rearrange("p (a b) -> p a b", a=2),
                                        in0=in0, in1=in1, op=A.mult)
                nc.vector.tensor_scalar(out=CACD[:, 5:6], in0=CACD[:, 5:6], scalar1=float(DIM),
                                        scalar2=None, op0=A.mult)
                nc.vector.tensor_scalar(out=CACD[:, 7:8], in0=CACD[:, 7:8], scalar1=float(DIM),
                                        scalar2=None, op0=A.mult)

                def cacd(k):
                    return bass.AP(tensor=CACD.tensor, offset=CACD.offset + k,
                                   ap=[list(CACD.ap[0]), [2, 2]])
                CA, CB, CC, CD = cacd(0), cacd(1), cacd(4), cacd(5)

                # per-head attention scalars -> uz6 [ud0 uo0 z0 ud1 uo1 z1]
                uz6 = wk.tile([128, 6], f32, name="uz6", tag="uz6")
                for h in range(2):
                    S4h = S[:, 4 * h:4 * h + 4]
                    kr2 = wk.tile([128, 2], f32, name=f"kr2_{h}", tag=f"kr2_{h}")
                    nc.vector.tensor_copy(out=kr2[:, 0:1], in_=S4h[:, 3:4])
                    nc.vector.tensor_copy(out=kr2[0:64, 1:2], in_=S4h[64:128, 3:4])
                    nc.vector.tensor_copy(out=kr2[64:128, 1:2], in_=S4h[0:64, 3:4])
                    sc = wk.tile([128, 2], f32, name=f"sc_{h}", tag=f"sc_{h}")
                    t3 = wk.tile([128, 2], f32, name=f"t3_{h}", tag=f"t3_{h}")
                    nc.vector.tensor_tensor(out=sc, in0=CA, in1=S4h[:, 0:2], op=A.mult)
                    nc.vector.scalar_tensor_tensor(out=sc, in0=CB, scalar=S4h[:, 2:3], in1=sc,
                                                   op0=A.mult, op1=A.add)
                    nc.vector.tensor_tensor(out=t3, in0=CC, in1=kr2, op=A.mult)
                    nc.vector.tensor_tensor(out=sc, in0=sc, in1=t3, op=A.add)
                    nc.vector.tensor_tensor(out=sc, in0=sc, in1=CD, op=A.add)
                    Dcol = wk.tile([128, 1], f32, name=f"D_{h}", tag=f"D_{h}")
                    Din = wk.tile([128, 1], f32, name=f"Di_{h}", tag=f"Di_{h}")
                    nc.vector.tensor_reduce(out=Dcol, in_=sc, axis=AX.X, op=A.add)
                    nc.vector.reciprocal(out=Din, in_=Dcol)
                    w2 = wk.tile([128, 2], f32, name=f"w2_{h}", tag=f"w2_{h}")
                    nc.vector.tensor_scalar(out=w2, in0=sc, scalar1=Din, scalar2=None, op0=A.mult)
                    t4 = wk.tile([128, 1], f32, name=f"t4_{h}", tag=f"t4_{h}")
                    nc.vector.tensor_scalar(out=uz6[:, 3 * h:3 * h + 1], in0=w2[:, 0:1],
                                            scalar1=bc[:, 2:3], scalar2=None, op0=A.mult)
                    nc.vector.tensor_scalar(out=uz6[:, 3 * h + 1:3 * h + 2], in0=w2[:, 1:2],
                                            scalar1=bc[:, 8:9], scalar2=None, op0=A.mult)
                    nc.vector.tensor_scalar(out=t4, in0=w2[:, 0:1], scalar1=bc[:, 5:6],
                                            scalar2=None, op0=A.mult)
                    nc.vector.scalar_tensor_tensor(out=uz6[:, 3 * h + 2:3 * h + 3], in0=w2[:, 1:2],
                                                   scalar=bc[:, 11:12], in1=t4,
                                                   op0=A.mult, op1=A.add)

                # base = z0*wos0 + z1*wos1 + bo  (no dependency on CC-B)
                base = wk.tile([128, 1024], f32, name="base", tag="base")
                nc.vector.scalar_tensor_tensor(out=base, in0=t_wos[:, 0, :], scalar=uz6[:, 2:3],
                                               in1=t_bo, op0=A.mult, op1=A.add)
                nc.vector.scalar_tensor_tensor(out=base, in0=t_wos[:, 1, :], scalar=uz6[:, 5:6],
                                               in1=base, op0=A.mult, op1=A.add)

                # ---- CC-B loadback + combine ----
                ys = wk.tile([128, 2048], f32, name="ys", tag="ys")
                if merged:
                    nc.sync.dma_start(out=ys[:, 0:1024], in_=ccm_out[pp][:, 0:1024])
                    nc.sync.dma_start(out=ys[:, 1024:2048], in_=ccm_out[pp][:, 1024:2048])
                elif fcc == f32:
                    nc.sync.dma_start(out=ys[:, 0:1024], in_=cc2_out[pp][:, 0:1024])
                    nc.sync.dma_start(out=ys[:, 1024:2048], in_=cc2_out[pp][:, 1024:2048])
                else:
                    ys16 = wk.tile([128, 2048], f16, name="ys16", tag="ys16")
                    nc.sync.dma_start(out=ys16[:, 0:1024], in_=cc2_out[pp][:, 0:1024])
                    nc.sync.dma_start(out=ys16[:, 1024:2048], in_=cc2_out[pp][:, 1024:2048])
                    nc.scalar.copy(out=ys[:, 0:1024], in_=ys16[:, 0:1024])
                    nc.scalar.copy(out=ys[:, 1024:2048], in_=ys16[:, 1024:2048])

                ysw = wk.tile([128, 2048], f32, name="ysw", tag="ysw")
                nc.vector.tensor_copy(out=ysw[0:64, :], in_=ys[64:128, :])
                nc.vector.tensor_copy(out=ysw[64:128, :], in_=ys[0:64, :])

                X = wk.tile([128, 1024], f32, name="X", tag="X")
                nc.vector.scalar_tensor_tensor(out=X, in0=ys[:, 0:1024], scalar=uz6[:, 0:1],
                                               in1=base, op0=A.mult, op1=A.add)
                nc.vector.scalar_tensor_tensor(out=X, in0=ys[:, 1024:2048], scalar=uz6[:, 3:4],
                                               in1=X, op0=A.mult, op1=A.add)
                nc.vector.scalar_tensor_tensor(out=X, in0=ysw[:, 0:1024], scalar=uz6[:, 1:2],
                                               in1=X, op0=A.mult, op1=A.add)
                nc.vector.scalar_tensor_tensor(out=X, in0=ysw[:, 1024:2048], scalar=uz6[:, 4:5],
                                               in1=X, op0=A.mult, op1=A.add)

                # ---- BN2 ----
                r2h = wk.tile([128, 2, 2], f32, name="r2h", tag="r2h")
                scr2 = wk.tile([128, 1024], f32, name="scr2", tag="scr2")
                for hh in range(2):
                    cols = slice(hh * 512, (hh + 1) * 512)
                    nc.vector.tensor_reduce(out=r2h[:, hh, 0:1], in_=X[:, cols], axis=AX.X, op=A.add)
                    nc.scalar.activation(out=scr2[:, cols], in_=X[:, cols], func=AF.Square,
                                         accum_out=r2h[:, hh, 1:2])
                r2 = wk.tile([128, 2], f32, name="r2", tag="r2")
                nc.vector.tensor_tensor(out=r2, in0=r2h[:, 0, :], in1=r2h[:, 1, :], op=A.add)
                st2_ps = ps.tile([128, 4], f32, name="st2", tag="sm")
                nc.tensor.matmul(out=st2_ps[0:2, 0:2], lhsT=t_mn, rhs=r2, start=True, stop=True)
                cst2 = wk.tile([2, 12], f32, name="cst2", tag="cst2")
                inv_n2 = 1.0 / float(B * DIM)
                nc.vector.tensor_scalar(out=cst2[:, 0:2], in0=st2_ps[0:2, 0:2], scalar1=inv_n2,
                                        scalar2=None, op0=A.mult)           # [mean, E2]
                nc.vector.tensor_tensor(out=cst2[:, 2:3], in0=cst2[:, 0:1], in1=cst2[:, 0:1], op=A.mult)
                nc.vector.tensor_tensor(out=cst2[:, 3:4], in0=cst2[:, 1:2], in1=cst2[:, 2:3], op=A.subtract)
                nc.scalar.activation(out=cst2[:, 4:5], in_=cst2[:, 3:4], func=AF.Sqrt,
                                     bias=eps_t, scale=1.0)
                nc.vector.reciprocal(out=cst2[:, 5:6], in_=cst2[:, 4:5])
                nc.vector.tensor_tensor(out=cst2[:, 6:7], in0=t_gb[0:2, 6:7], in1=cst2[:, 5:6], op=A.mult)  # abn
                nc.vector.tensor_tensor(out=cst2[:, 8:9], in0=cst2[:, 6:7], in1=cst2[:, 0:1], op=A.mult)
                nc.vector.tensor_tensor(out=cst2[:, 7:8], in0=t_gb[0:2, 7:8], in1=cst2[:, 8:9], op=A.subtract)  # cbn
                bc2_ps = ps.tile([128, 4], f32, name="bc2", tag="vr")
                nc.tensor.matmul(out=bc2_ps[:, 0:2], lhsT=t_m2, rhs=cst2[:, 6:8], start=True, stop=True)
                bc2 = wk.tile([128, 2], f32, name="bc2_sb", tag="bc2_sb")
                nc.scalar.copy(out=bc2, in_=bc2_ps[:, 0:2])
                fin = wk.tile([128, 1024], f32, name="fin", tag="fin")
                for hh in range(2):
                    cols = slice(hh * 512, (hh + 1) * 512)
                    nc.vector.tensor_scalar(out=fin[:, cols], in0=X[:, cols], scalar1=bc2[:, 0:1],
                                            scalar2=bc2[:, 1:2], op0=A.mult, op1=A.add)
                    nc.sync.dma_start(out=d_out[:, cols], in_=fin[:, cols])

    nc.compile()
    return nc


# --------------------------------------------------------------------------
# v6: software-pipelined emission — phase1(k+1) is emitted before phase2(k)
# so no engine queue ever blocks the next rep's collective inputs behind the
# current rep's post-collective tail.
# --------------------------------------------------------------------------

def _build_v6(mm_dt=None, reps=1, packed=False, slim=False, act_pack=False):
    import concourse.bass as bass
    import concourse.mybir as mybir
    import concourse.tile as tile
    from concourse import bacc

    f32 = mybir.dt.float32
    f16 = mybir.dt.float16
    i16 = mybir.dt.int16
    fmm = mybir.dt.float32r if (mm_dt or MM_DT) == "f32r" else mybir.dt.float32
    fcc = f32 if CC_F32 else f16
    A = mybir.AluOpType
    AF = mybir.ActivationFunctionType
    AX = mybir.AxisListType

    no_cc = NO_CC
    nc = bacc.Bacc(None, target_bir_lowering=False, debug=False, num_devices=NC)

    d_xt = nc.dram_tensor("xt", [128, 8, 128], fmm, kind="ExternalInput")
    d_wqk = nc.dram_tensor("wqk", [128, 8, 512], fmm, kind="ExternalInput")
    d_wv = nc.dram_tensor("wv", [128, 8, 256], f16, kind="ExternalInput")
    d_wo = nc.dram_tensor("wo", [128, 2, 1024], f16, kind="ExternalInput")
    d_wos = nc.dram_tensor("wos", [2, 1024], f32, kind="ExternalInput")
    d_bo = nc.dram_tensor("bo", [1024], f32, kind="ExternalInput")
    d_hm = nc.dram_tensor("hm", [128, 2], f32, kind="ExternalInput")
    d_mn = nc.dram_tensor("mn", [128, 2], f32, kind="ExternalInput")
    d_m2 = nc.dram_tensor("m2", [2, 128], f32, kind="ExternalInput")
    d_gb = nc.dram_tensor("gb", [2, 8], f32, kind="ExternalInput")
    d_out = nc.dram_tensor("out", [128, 1024], f32, kind="ExternalOutput")

    NCC = 2
    if packed:
        d_psc = nc.dram_tensor("psc", [128, 16], f32, kind="ExternalInput")
        d_ipsc = nc.dram_tensor("ipsc", [128, 16], f32, kind="ExternalInput")
        d_hmy = nc.dram_tensor("hmy", [128, 2], f32, kind="ExternalInput")
        d_isy = nc.dram_tensor("isy", [128, 1], f32, kind="ExternalInput")
        ccp_in = [nc.dram_tensor(f"ccp_in{i}", [128, 2080], i16) for i in range(NCC)]
        ccp_out = [nc.dram_tensor(f"ccp_out{i}", [128, 2080], i16, addr_space="Shared")
                   for i in range(NCC)]
    else:
        cc1_in = [nc.dram_tensor(f"cc1_in{i}", [128, 16], f32) for i in range(NCC)]
        cc1_out = [nc.dram_tensor(f"cc1_out{i}", [128, 16], f32, addr_space="Shared")
                   for i in range(NCC)]
        cc2_in = [nc.dram_tensor(f"cc2_in{i}", [128, 2048], fcc) for i in range(NCC)]
        cc2_out = [nc.dram_tensor(f"cc2_out{i}", [128, 2048], fcc, addr_space="Shared")
                   for i in range(NCC)]

    def bcast(ap, p=128):
        return bass.AP(tensor=ap.tensor, offset=ap.offset, ap=[[0, p]] + list(ap.ap))

    groups = [list(range(NC))]

    with tile.TileContext(nc) as tc:
        with (
            tc.tile_pool(name="const", bufs=2) as cst_pool,
            tc.tile_pool(name="work", bufs=1) as wk,
            tc.tile_pool(name="psum", bufs=1, space="PSUM") as ps,
            tc.tile_pool(name="psvt", bufs=2, space="PSUM") as psvt,
        ):
            def phase1(rep):
                pp = rep % NCC
                t_xt = [cst_pool.tile([128, 128], fmm, name=f"xt{c}", tag=f"xt{c}") for c in range(8)]
                t_wqk = [cst_pool.tile([128, 512], fmm, name=f"wqk{c}", tag=f"wqk{c}") for c in range(8)]
                t_wv = [cst_pool.tile([128, 256], f16, name=f"wv{c}", tag=f"wv{c}") for c in range(8)]
                t_wo = [cst_pool.tile([128, 1024], f16, name=f"wo{k}", tag=f"wo{k}") for k in range(2)]
                t_wos = cst_pool.tile([128, 2, 1024], f32, name="wos", tag="wos")
                t_bo = cst_pool.tile([128, 1024], f32, name="bo", tag="bo")
                t_hm = cst_pool.tile([128, 2], f32, name="hm", tag="hm")
                t_mn = cst_pool.tile([128, 2], f32, name="mn", tag="mn")
                t_m2 = cst_pool.tile([2, 128], f32, name="m2", tag="m2")
                t_gb = cst_pool.tile([2, 8], f32, name="gb", tag="gb")

                for c in range(8):
                    nc.sync.dma_start(out=t_xt[c], in_=d_xt[:, c, :])
                    nc.sync.dma_start(out=t_wqk[c], in_=d_wqk[:, c, :])
                for c in range(8):
                    nc.sync.dma_start(out=t_wv[c], in_=d_wv[:, c, :])
                for k in range(2):
                    nc.sync.dma_start(out=t_wo[k], in_=d_wo[:, k, :])
                nc.sync.dma_start(out=t_hm, in_=d_hm[:])
                nc.sync.dma_start(out=t_mn, in_=d_mn[:])
                nc.sync.dma_start(out=t_m2, in_=d_m2[:])
                nc.sync.dma_start(out=t_gb, in_=d_gb[:])
                nc.sync.dma_start(out=t_wos, in_=bcast(d_wos[:]))
                nc.sync.dma_start(out=t_bo, in_=bcast(d_bo[:]))
                if packed:
                    t_psc = cst_pool.tile([128, 16], f32, name="psc", tag="psc")
                    t_ipsc = cst_pool.tile([128, 16], f32, name="ipsc", tag="ipsc")
                    t_hmy = cst_pool.tile([128, 2], f32, name="hmy", tag="hmy")
                    t_isy = cst_pool.tile([128, 1], f32, name="isy", tag="isy")
                    nc.sync.dma_start(out=t_psc, in_=d_psc[:])
                    nc.sync.dma_start(out=t_ipsc, in_=d_ipsc[:])
                    nc.sync.dma_start(out=t_hmy, in_=d_hmy[:])
                    nc.sync.dma_start(out=t_isy, in_=d_isy[:])

                t_x16 = [wk.tile([128, 128], f16, name=f"x16_{c}", tag=f"x16_{c}", bufs=2)
                         for c in range(8)]
                for c in range(8):
                    nc.scalar.copy(out=t_x16[c], in_=t_xt[c])

                qk_ps = ps.tile([128, 512], f32, name="qk", tag="qk", bufs=2)
                for c in range(8):
                    nc.tensor.matmul(qk_ps, lhsT=t_xt[c], rhs=t_wqk[c],
                                     start=(c == 0), stop=(c == 7))
                vt_ps = [psvt.tile([128, 128], f32, name="vt", tag="vt") for _ in range(2)]
                for half in range(2):
                    for c in range(8):
                        nc.tensor.matmul(vt_ps[half], lhsT=t_wv[c][:, half * 128:(half + 1) * 128],
                                         rhs=t_x16[c], start=(c == 0), stop=(c == 7))

                # payload
                q_ap = qk_ps[:, 0:256]
                k_ap = qk_ps[:, 256:512]
                ksb = wk.tile([128, 256], f32, name="ksb", tag="ksb", bufs=2)
                nc.scalar.copy(out=ksb, in_=k_ap)
                ksw = wk.tile([128, 256], f32, name="ksw", tag="ksw", bufs=2)
                nc.vector.tensor_copy(out=ksw[0:64, :], in_=ksb[64:128, :])
                nc.vector.tensor_copy(out=ksw[64:128, :], in_=ksb[0:64, :])

                tmp4 = wk.tile([128, 4], f32, name="tmp4", tag="tmp4", bufs=2)
                prod1 = wk.tile([128, 256], f32, name="prod1", tag="prod1", bufs=2)
                prod2 = wk.tile([128, 256], f32, name="prod2", tag="prod2", bufs=2)
                nc.vector.tensor_tensor(out=prod1, in0=q_ap, in1=ksb, op=A.mult)
                nc.vector.tensor_reduce(out=tmp4[:, 0:1], in_=prod1, axis=AX.X, op=A.add)
                nc.vector.tensor_tensor(out=prod2, in0=q_ap, in1=ksw, op=A.mult)
                nc.vector.tensor_reduce(out=tmp4[:, 1:2], in_=prod2, axis=AX.X, op=A.add)
                nc.vector.tensor_reduce(out=tmp4[:, 2:4],
                                        in_=qk_ps[:].rearrange("p (t j) -> p t j", t=2),
                                        axis=AX.X, op=A.add)

                pay = wk.tile([128, 16], f32, name="pay", tag="pay", bufs=2)
                nc.vector.memset(pay, 0.0)
                nc.vector.tensor_scalar(out=pay[:, 0:4], in0=tmp4, scalar1=t_hm[:, 0:1],
                                        scalar2=None, op0=A.mult)
                nc.vector.tensor_scalar(out=pay[:, 4:8], in0=tmp4, scalar1=t_hm[:, 1:2],
                                        scalar2=None, op0=A.mult)
                sq1 = wk.tile([128, 256], f32, name="sq1", tag="sq1", bufs=2)
                sq2 = wk.tile([128, 256], f32, name="sq2", tag="sq2", bufs=2)
                nc.scalar.activation(out=sq1, in_=q_ap, func=AF.Square, accum_out=pay[:, 8:9])
                nc.scalar.activation(out=sq2, in_=k_ap, func=AF.Square, accum_out=pay[:, 9:10])
                vsq = [wk.tile([128, 128], f32, name=f"vsq{i}", tag=f"vsq{i}", bufs=2) for i in range(2)]
                v2ab = [wk.tile([128, 2], f32, name=f"v2ab{i}", tag=f"v2ab{i}", bufs=2) for i in range(2)]
                for half in range(2):
                    nc.scalar.activation(out=vsq[half], in_=vt_ps[half], func=AF.Square)
                    nc.vector.tensor_reduce(out=v2ab[half],
                                            in_=vsq[half][:].rearrange("p (t r) -> p t r", t=2),
                                            axis=AX.X, op=A.add)
                nc.vector.tensor_tensor(out=pay[:, 11:13], in0=v2ab[0], in1=v2ab[1], op=A.add)
                vsab = [wk.tile([128, 2], f32, name=f"vsab{i}", tag=f"vsab{i}", bufs=2) for i in range(2)]
                for half in range(2):
                    nc.vector.tensor_reduce(out=vsab[half],
                                            in_=vt_ps[half][:].rearrange("p (t r) -> p t r", t=2),
                                            axis=AX.X, op=A.add)
                nc.vector.tensor_tensor(out=pay[:, 13:15], in0=vsab[0], in1=vsab[1], op=A.add)

                if packed:
                    # fixed-point encode: v*s = hi*2048 + lo, exact int adds on wire
                    ps1 = wk.tile([128, 16], f32, name="ps1", tag="ps1", bufs=2)
                    nc.vector.tensor_tensor(out=ps1, in0=pay, in1=t_psc, op=A.mult)
                    payi = wk.tile([128, 32], i16, name="payi", tag="payi", bufs=2)
                    nc.vector.tensor_scalar(out=payi[:, 0:16], in0=ps1, scalar1=1.0 / 2048.0,
                                            scalar2=None, op0=A.mult)
                    hi_f = wk.tile([128, 16], f32, name="hi_f", tag="hi_f", bufs=2)
                    nc.vector.tensor_copy(out=hi_f, in_=payi[:, 0:16])
                    nc.vector.scalar_tensor_tensor(out=payi[:, 16:32], in0=hi_f, scalar=-2048.0,
                                                   in1=ps1, op0=A.mult, op1=A.add)
                    nc.sync.dma_start(out=ccp_in[pp][:, 2048:2080], in_=payi)
                else:
                    # CC-A issued as soon as the payload is staged
                    nc.sync.dma_start(out=cc1_in[pp][:], in_=pay)
                    if no_cc in (True, "no_a"):
                        nc.gpsimd.dma_start(out=cc1_out[pp][:], in_=cc1_in[pp][:])
                    else:
                        nc.gpsimd.collective_compute(
                            "AllReduce", A.add, replica_groups=groups,
                            ins=[cc1_in[pp][:]], outs=[cc1_out[pp][:]])

                # Y matmuls + head-pack + CC-B
                vts = wk.tile([128, 2, 128], f16, name="vts", tag="vts", bufs=2)
                for half in range(2):
                    nc.scalar.copy(out=vts[:, half, :], in_=vt_ps[half])
                y_ps = ps.tile([128, 1024], f32, name="y", tag="y")
                for k in range(2):
                    for nn in range(2):
                        nc.tensor.matmul(y_ps[:, nn * 512:(nn + 1) * 512],
                                         lhsT=vts[:, k, :], rhs=t_wo[k][:, nn * 512:(nn + 1) * 512],
                                         start=(k == 0), stop=(k == 1))
                if packed:
                    yh = wk.tile([128, 2048], i16, name="yh", tag="yh", bufs=2)
                    if act_pack:
                        nc.scalar.activation(out=yh[:, 0:1024], in_=y_ps, func=AF.Identity,
                                             scale=t_hmy[:, 0:1])
                        nc.scalar.activation(out=yh[:, 1024:2048], in_=y_ps, func=AF.Identity,
                                             scale=t_hmy[:, 1:2])
                    else:
                        nc.vector.tensor_scalar(out=yh[:, 0:1024], in0=y_ps, scalar1=t_hmy[:, 0:1],
                                                scalar2=None, op0=A.mult)
                        nc.vector.tensor_scalar(out=yh[:, 1024:2048], in0=y_ps, scalar1=t_hmy[:, 1:2],
                                                scalar2=None, op0=A.mult)
                    nc.sync.dma_start(out=ccp_in[pp][:, 0:2048], in_=yh)
                    if no_cc:
                        nc.gpsimd.dma_start(out=ccp_out[pp][:], in_=ccp_in[pp][:])
                    else:
                        nc.gpsimd.collective_compute(
                            "AllReduce", A.add, replica_groups=groups,
                            ins=[ccp_in[pp][:]], outs=[ccp_out[pp][:]],
                            cc_dim=CC_DIM, unique_tensors=CC_UT)
                else:
                    yh = wk.tile([128, 2048], fcc, name="yh", tag="yh", bufs=2)
                    nc.vector.tensor_scalar(out=yh[:, 0:1024], in0=y_ps, scalar1=t_hm[:, 0:1],
                                            scalar2=None, op0=A.mult)
                    nc.vector.tensor_scalar(out=yh[:, 1024:2048], in0=y_ps, scalar1=t_hm[:, 1:2],
                                            scalar2=None, op0=A.mult)
                    nc.sync.dma_start(out=cc2_in[pp][:], in_=yh)
                    if no_cc in (True, "no_b"):
                        nc.gpsimd.dma_start(out=cc2_out[pp][:], in_=cc2_in[pp][:])
                    else:
                        nc.gpsimd.collective_compute(
                            "AllReduce", A.add, replica_groups=groups,
                            ins=[cc2_in[pp][:]], outs=[cc2_out[pp][:]])

                ctx = dict(pp=pp, t_wos=t_wos, t_bo=t_bo, t_mn=t_mn, t_m2=t_m2,
                           t_gb=t_gb)
                if packed:
                    ctx.update(t_ipsc=t_ipsc, t_isy=t_isy)
                return ctx

            def phase2(ctx):
                pp = ctx["pp"]
                t_wos, t_bo = ctx["t_wos"], ctx["t_bo"]
                t_mn, t_m2, t_gb = ctx["t_mn"], ctx["t_m2"], ctx["t_gb"]

                # loadbacks (emitted after phase1 of the NEXT rep)
                S = wk.tile([128, 16], f32, name="S", tag="S", bufs=2)
                ys = wk.tile([128, 2048], f32, name="ys", tag="ys")
                if packed:
                    t_ipsc, t_isy = ctx["t_ipsc"], ctx["t_isy"]
                    Si = wk.tile([128, 32], i16, name="Si", tag="Si", bufs=2)
                    nc.sync.dma_start(out=Si, in_=ccp_out[pp][:, 2048:2080])
                    Shi = wk.tile([128, 16], f32, name="Shi", tag="Shi")
                    Slo = wk.tile([128, 16], f32, name="Slo", tag="Slo")
                    nc.vector.tensor_copy(out=Shi, in_=Si[:, 0:16])
                    nc.vector.tensor_copy(out=Slo, in_=Si[:, 16:32])
                    nc.vector.scalar_tensor_tensor(out=Slo, in0=Shi, scalar=2048.0, in1=Slo,
                                                   op0=A.mult, op1=A.add)
                    nc.vector.tensor_tensor(out=S, in0=Slo, in1=t_ipsc, op=A.mult)
                    ysi = wk.tile([128, 2048], i16, name="ysi", tag="ysi", bufs=2)
                    nc.sync.dma_start(out=ysi[:, 0:1024], in_=ccp_out[pp][:, 0:1024])
                    nc.sync.dma_start(out=ysi[:, 1024:2048], in_=ccp_out[pp][:, 1024:2048])
                    if not slim:
                        nc.vector.tensor_scalar(out=ys[:, 0:1024], in0=ysi[:, 0:1024],
                                                scalar1=t_isy[:, 0:1], scalar2=None, op0=A.mult)
                        nc.vector.tensor_scalar(out=ys[:, 1024:2048], in0=ysi[:, 1024:2048],
                                                scalar1=t_isy[:, 0:1], scalar2=None, op0=A.mult)
                elif fcc == f16:
                    nc.sync.dma_start(out=S, in_=cc1_out[pp][:])
                    ys16 = wk.tile([128, 2048], f16, name="ys16", tag="ys16", bufs=2)
                    nc.sync.dma_start(out=ys16[:, 0:1024], in_=cc2_out[pp][:, 0:1024])
                    nc.sync.dma_start(out=ys16[:, 1024:2048], in_=cc2_out[pp][:, 1024:2048])
                    nc.scalar.copy(out=ys[:, 0:1024], in_=ys16[:, 0:1024])
                    nc.scalar.copy(out=ys[:, 1024:2048], in_=ys16[:, 1024:2048])
                else:
                    nc.sync.dma_start(out=S, in_=cc1_out[pp][:])
                    nc.sync.dma_start(out=ys[:, 0:1024], in_=cc2_out[pp][:, 0:1024])
                    nc.sync.dma_start(out=ys[:, 1024:2048], in_=cc2_out[pp][:, 1024:2048])

                # BN1 stats
                rhs4 = wk.tile([128, 4], f32, name="rhs4", tag="rhs4")
                nc.vector.tensor_tensor(out=rhs4[:, 0:2], in0=S[:, 2:4], in1=S[:, 6:8], op=A.add)
                nc.vector.tensor_copy(out=rhs4[:, 2:4], in_=S[:, 8:10])
                ones1 = wk.tile([128, 1], f32, name="ones1", tag="ones1")
                nc.vector.memset(ones1, 1.0)
                st_ps = ps.tile([128, 16], f32, name="st", tag="sm")
                nc.tensor.matmul(out=st_ps[0:2, 0:4], lhsT=t_mn, rhs=rhs4, start=True, stop=True)
                nc.tensor.matmul(out=st_ps[0:2, 4:5], lhsT=S[:, 11:13], rhs=ones1,
                                 start=True, stop=True)
                nc.tensor.matmul(out=st_ps[0:2, 5:6], lhsT=S[:, 13:15], rhs=ones1,
                                 start=True, stop=True)
                sts = wk.tile([2, 6], f32, name="sts", tag="sts")
                nc.vector.tensor_copy(out=sts[:, 0:2], in_=st_ps[0:2, 0:2])
                nc.vector.tensor_copy(out=sts[:, 2:3], in_=st_ps[0:2, 5:6])
                nc.vector.tensor_copy(out=sts[:, 3:5], in_=st_ps[0:2, 2:4])
                nc.vector.tensor_copy(out=sts[:, 5:6], in_=st_ps[0:2, 4:5])

                cst = wk.tile([2, 32], f32, name="cst", tag="cst")
                eps_t = wk.tile([2, 1], f32, name="eps_t", tag="eps_t")
                nc.vector.memset(eps_t, EPS)
                inv_n1 = 1.0 / float(B * INNER)
                nc.vector.tensor_scalar(out=cst[:, 0:3], in0=sts[:, 0:3], scalar1=inv_n1,
                                        scalar2=None, op0=A.mult)
                nc.vector.tensor_scalar(out=cst[:, 3:6], in0=sts[:, 3:6], scalar1=inv_n1,
                                        scalar2=None, op0=A.mult)
                nc.vector.tensor_tensor(out=cst[:, 6:9], in0=cst[:, 0:3], in1=cst[:, 0:3], op=A.mult)
                nc.vector.tensor_tensor(out=cst[:, 9:12], in0=cst[:, 3:6], in1=cst[:, 6:9], op=A.subtract)
                nc.scalar.activation(out=cst[:, 12:15], in_=cst[:, 9:12], func=AF.Sqrt,
                                     bias=eps_t, scale=1.0)
                nc.vector.reciprocal(out=cst[:, 15:18], in_=cst[:, 12:15])
                nc.vector.tensor_tensor(out=cst[:, 18:21], in0=t_gb[0:2, 0:3], in1=cst[:, 15:18],
                                        op=A.mult)
                nc.vector.tensor_tensor(out=cst[:, 24:27], in0=cst[:, 18:21], in1=cst[:, 0:3],
                                        op=A.mult)
                nc.vector.tensor_tensor(out=cst[:, 21:24], in0=t_gb[0:2, 3:6], in1=cst[:, 24:27],
                                        op=A.subtract)

                bc_ps = ps.tile([128, 16], f32, name="bc", tag="sm")
                nc.tensor.matmul(out=bc_ps[:, 0:6], lhsT=t_m2, rhs=cst[:, 18:24],
                                 start=True, stop=True)
                bc = wk.tile([128, 12], f32, name="bc_sb", tag="bc_sb")
                nc.scalar.copy(out=bc[:, 0:6], in_=bc_ps[:, 0:6])
                nc.vector.tensor_copy(out=bc[0:64, 6:12], in_=bc[64:128, 0:6])
                nc.vector.tensor_copy(out=bc[64:128, 6:12], in_=bc[0:64, 0:6])

                CACD = wk.tile([128, 8], f32, name="CACD", tag="CACD")
                in0 = bass.AP(tensor=bc.tensor, offset=bc.offset,
                              ap=[list(bc.ap[0]), [3, 2], [0, 4]])
                in1 = bass.AP(tensor=bc.tensor, offset=bc.offset + 1,
                              ap=[list(bc.ap[0]), [0, 2], [3, 4]])
                nc.vector.tensor_tensor(out=CACD[:].rearrange("p (a b) -> p a b", a=2),
                                        in0=in0, in1=in1, op=A.mult)
                nc.vector.tensor_scalar(out=CACD[:, 5:6], in0=CACD[:, 5:6], scalar1=float(DIM),
                                        scalar2=None, op0=A.mult)
                nc.vector.tensor_scalar(out=CACD[:, 7:8], in0=CACD[:, 7:8], scalar1=float(DIM),
                                        scalar2=None, op0=A.mult)

                def cacd(k):
                    return bass.AP(tensor=CACD.tensor, offset=CACD.offset + k,
                                   ap=[list(CACD.ap[0]), [2, 2]])
                CA, CB, CC, CD = cacd(0), cacd(1), cacd(4), cacd(5)

                uz6 = wk.tile([128, 6], f32, name="uz6", tag="uz6")
                for h in range(2):
                    S4h = S[:, 4 * h:4 * h + 4]
                    kr2 = wk.tile([128, 2], f32, name=f"kr2_{h}", tag=f"kr2_{h}")
                    nc.vector.tensor_copy(out=kr2[:, 0:1], in_=S4h[:, 3:4])
                    nc.vector.tensor_copy(out=kr2[0:64, 1:2], in_=S4h[64:128, 3:4])
                    nc.vector.tensor_copy(out=kr2[64:128, 1:2], in_=S4h[0:64, 3:4])
                    sc = wk.tile([128, 2], f32, name=f"sc_{h}", tag=f"sc_{h}")
                    t3 = wk.tile([128, 2], f32, name=f"t3_{h}", tag=f"t3_{h}")
                    nc.vector.tensor_tensor(out=sc, in0=CA, in1=S4h[:, 0:2], op=A.mult)
                    nc.vector.scalar_tensor_tensor(out=sc, in0=CB, scalar=S4h[:, 2:3], in1=sc,
                                                   op0=A.mult, op1=A.add)
                    nc.vector.tensor_tensor(out=t3, in0=CC, in1=kr2, op=A.mult)
                    nc.vector.tensor_tensor(out=sc, in0=sc, in1=t3, op=A.add)
                    nc.vector.tensor_tensor(out=sc, in0=sc, in1=CD, op=A.add)
                    Dcol = wk.tile([128, 1], f32, name=f"D_{h}", tag=f"D_{h}")
                    Din = wk.tile([128, 1], f32, name=f"Di_{h}", tag=f"Di_{h}")
                    nc.vector.tensor_reduce(out=Dcol, in_=sc, axis=AX.X, op=A.add)
                    nc.vector.reciprocal(out=Din, in_=Dcol)
                    w2 = wk.tile([128, 2], f32, name=f"w2_{h}", tag=f"w2_{h}")
                    nc.vector.tensor_scalar(out=w2, in0=sc, scalar1=Din, scalar2=None, op0=A.mult)
                    t4 = wk.tile([128, 1], f32, name=f"t4_{h}", tag=f"t4_{h}")
                    nc.vector.tensor_scalar(out=uz6[:, 3 * h:3 * h + 1], in0=w2[:, 0:1],
                                            scalar1=bc[:, 2:3], scalar2=None, op0=A.mult)
                    nc.vector.tensor_scalar(out=uz6[:, 3 * h + 1:3 * h + 2], in0=w2[:, 1:2],
                                            scalar1=bc[:, 8:9], scalar2=None, op0=A.mult)
                    nc.vector.tensor_scalar(out=t4, in0=w2[:, 0:1], scalar1=bc[:, 5:6],
                                            scalar2=None, op0=A.mult)
                    nc.vector.scalar_tensor_tensor(out=uz6[:, 3 * h + 2:3 * h + 3], in0=w2[:, 1:2],
                                                   scalar=bc[:, 11:12], in1=t4,
                                                   op0=A.mult, op1=A.add)

                base = wk.tile([128, 1024], f32, name="base", tag="base")
                nc.vector.scalar_tensor_tensor(out=base, in0=t_wos[:, 0, :], scalar=uz6[:, 2:3],
                                               in1=t_bo, op0=A.mult, op1=A.add)
                nc.vector.scalar_tensor_tensor(out=base, in0=t_wos[:, 1, :], scalar=uz6[:, 5:6],
                                               in1=base, op0=A.mult, op1=A.add)

                if packed and slim:
                    # fold 1/sY into the per-row Y scalars; combine straight
                    # from int16 with converting reads
                    nc.vector.tensor_scalar(out=uz6[:, 0:2], in0=uz6[:, 0:2],
                                            scalar1=t_isy[:, 0:1], scalar2=None, op0=A.mult)
                    nc.vector.tensor_scalar(out=uz6[:, 3:5], in0=uz6[:, 3:5],
                                            scalar1=t_isy[:, 0:1], scalar2=None, op0=A.mult)
                    ysrc = ysi
                    ysw = wk.tile([128, 2048], i16, name="ysw", tag="ysw")
                else:
                    ysrc = ys
                    ysw = wk.tile([128, 2048], f32, name="ysw", tag="ysw")
                nc.vector.tensor_copy(out=ysw[0:64, :], in_=ysrc[64:128, :])
                nc.vector.tensor_copy(out=ysw[64:128, :], in_=ysrc[0:64, :])

                X = wk.tile([128, 1024], f32, name="X", tag="X")
                nc.vector.scalar_tensor_tensor(out=X, in0=ysrc[:, 0:1024], scalar=uz6[:, 0:1],
                                               in1=base, op0=A.mult, op1=A.add)
                nc.vector.scalar_tensor_tensor(out=X, in0=ysrc[:, 1024:2048], scalar=uz6[:, 3:4],
                                               in1=X, op0=A.mult, op1=A.add)
                nc.vector.scalar_tensor_tensor(out=X, in0=ysw[:, 0:1024], scalar=uz6[:, 1:2],
                                               in1=X, op0=A.mult, op1=A.add)
                nc.vector.scalar_tensor_tensor(out=X, in0=ysw[:, 1024:2048], scalar=uz6[:, 4:5],
                                               in1=X, op0=A.mult, op1=A.add)

                # BN2
                r2h = wk.tile([128, 2, 2], f32, name="r2h", tag="r2h")
                scr2 = wk.tile([128, 1024], f32, name="scr2", tag="scr2")
                for hh in range(2):
                    cols = slice(hh * 512, (hh + 1) * 512)
                    nc.vector.tensor_reduce(out=r2h[:, hh, 0:1], in_=X[:, cols], axis=AX.X, op=A.add)
                    nc.scalar.activation(out=scr2[:, cols], in_=X[:, cols], func=AF.Square,
                                         accum_out=r2h[:, hh, 1:2])
                r2 = wk.tile([128, 2], f32, name="r2", tag="r2")
                nc.vector.tensor_tensor(out=r2, in0=r2h[:, 0, :], in1=r2h[:, 1, :], op=A.add)
                st2_ps = ps.tile([128, 4], f32, name="st2", tag="sm")
                nc.tensor.matmul(out=st2_ps[0:2, 0:2], lhsT=t_mn, rhs=r2, start=True, stop=True)
                cst2 = wk.tile([2, 12], f32, name="cst2", tag="cst2")
                inv_n2 = 1.0 / float(B * DIM)
                nc.vector.tensor_scalar(out=cst2[:, 0:2], in0=st2_ps[0:2, 0:2], scalar1=inv_n2,
                                        scalar2=None, op0=A.mult)
                nc.vector.tensor_tensor(out=cst2[:, 2:3], in0=cst2[:, 0:1], in1=cst2[:, 0:1], op=A.mult)
                nc.vector.tensor_tensor(out=cst2[:, 3:4], in0=cst2[:, 1:2], in1=cst2[:, 2:3], op=A.subtract)
                nc.scalar.activation(out=cst2[:, 4:5], in_=cst2[:, 3:4], func=AF.Sqrt,
                                     bias=eps_t, scale=1.0)
                nc.vector.reciprocal(out=cst2[:, 5:6], in_=cst2[:, 4:5])
                nc.vector.tensor_tensor(out=cst2[:, 6:7], in0=t_gb[0:2, 6:7], in1=cst2[:, 5:6], op=A.mult)
                nc.vector.tensor_tensor(out=cst2[:, 8:9], in0=cst2[:, 6:7], in1=cst2[:, 0:1], op=A.mult)
                nc.vector.tensor_tensor(out=cst2[:, 7:8], in0=t_gb[0:2, 7:8], in1=cst2[:, 8:9], op=A.subtract)
                bc2_ps = ps.tile([128, 4], f32, name="bc2", tag="sm")
                nc.tensor.matmul(out=bc2_ps[:, 0:2], lhsT=t_m2, rhs=cst2[:, 6:8], start=True, stop=True)
                bc2 = wk.tile([128, 2], f32, name="bc2_sb", tag="bc2_sb")
                nc.scalar.copy(out=bc2, in_=bc2_ps[:, 0:2])
                fin = wk.tile([128, 1024], f32, name="fin", tag="fin")
                for hh in range(2):
                    cols = slice(hh * 512, (hh + 1) * 512)
                    if slim:
                        nc.scalar.activation(out=fin[:, cols], in_=X[:, cols], func=AF.Identity,
                                             scale=bc2[:, 0:1], bias=bc2[:, 1:2])
                    else:
                        nc.vector.tensor_scalar(out=fin[:, cols], in0=X[:, cols], scalar1=bc2[:, 0:1],
                                                scalar2=bc2[:, 1:2], op0=A.mult, op1=A.add)
                    nc.sync.dma_start(out=d_out[:, cols], in_=fin[:, cols])

            ctxs = []
            for rep in range(reps):
                ctxs.append(phase1(rep))
                if rep >= 1:
                    phase2(ctxs[rep - 1])
            phase2(ctxs[-1])

    nc.compile()
    return nc


def _prep_inputs_v2(x, Wq, Wk, Wv, Wo, bo, g_q, b_q, g_k, b_k, g_v, b_v, g_bn, b_bn):
    f = np.float32
    f16 = np.float16
    x, Wq, Wk, Wv, Wo, bo = (np.asarray(t, f) for t in (x, Wq, Wk, Wv, Wo, bo))
    g_q, b_q, g_k, b_k, g_v, b_v, g_bn, b_bn = (
        np.asarray(t, f) for t in (g_q, b_q, g_k, b_k, g_v, b_v, g_bn, b_bn))
    xf = np.ascontiguousarray(x, f).reshape(B, N, DIM)
    Xr = np.ascontiguousarray(xf.transpose(1, 0, 2).reshape(N * B, DIM))   # n-major rows
    xt = np.ascontiguousarray(Xr.T.reshape(8, 128, 128).transpose(1, 0, 2))  # [p, c, r]

    mn = np.zeros((128, 2), f)
    mn[0:64, 0] = 1.0
    mn[64:128, 1] = 1.0
    m2 = np.ascontiguousarray(mn.T)            # (2, 128)
    gb = np.stack([g_q, g_k, g_v, b_q, b_k, b_v, g_bn, b_bn], axis=1).astype(f)
    wos = np.stack([Wo[:, 0:DIM].sum(1), Wo[:, DIM:INNER].sum(1)]).astype(f)  # (2, 1024)

    in_maps = []
    for i in range(NC):
        rows = slice(i * DPC, (i + 1) * DPC)
        head = i // 4
        wqk_c = np.concatenate([Wq[rows], Wk[rows]], axis=0).astype(f)       # (512, 1024)
        wqk = np.ascontiguousarray(wqk_c.T.reshape(8, 128, 512).transpose(1, 0, 2))
        wv_c = np.asarray(Wv[rows], f16)                                      # (256, 1024)
        wv = np.ascontiguousarray(wv_c.T.reshape(8, 128, 256).transpose(1, 0, 2))
        WoC = np.asarray(Wo[:, rows], f16)                                    # (1024, 256)
        wo = np.ascontiguousarray(WoC.T.reshape(2, 128, 1024).transpose(1, 0, 2))
        hm = np.zeros((128, 2), f)
        hm[:, head] = 1.0
        in_maps.append({
            "xt": xt, "wqk": wqk, "wv": wv, "wo": wo,
            "wos": wos, "bo": bo, "hm": hm, "mn": mn, "m2": m2, "gb": gb,
        })
    return in_maps


# --------------------------------------------------------------------------
# v1 (legacy, known-good): serial AllGather -> combine -> AllReduce, all f32
# --------------------------------------------------------------------------

def _build_v1(mm_dt=None, reps=1):
    import concourse.bass as bass
    import concourse.mybir as mybir
    import concourse.tile as tile
    from concourse import bacc

    f32 = mybir.dt.float32
    fmm = mybir.dt.float32r if (mm_dt or MM_DT) == "f32r" else mybir.dt.float32
    A = mybir.AluOpType
    AF = mybir.ActivationFunctionType
    AX = mybir.AxisListType

    no_cc = NO_CC
    nc = bacc.Bacc(None, target_bir_lowering=False, debug=False, num_devices=NC)

    # ---- I/O ----
    d_xt = nc.dram_tensor("xt", [128, 8, 128], fmm, kind="ExternalInput")
    d_wqk = nc.dram_tensor("wqk", [128, 8, 512], fmm, kind="ExternalInput")
    d_wv = nc.dram_tensor("wv", [128, 8, 256], fmm, kind="ExternalInput")
    d_wo = nc.dram_tensor("wo", [128, 2, 1024], fmm, kind="ExternalInput")
    d_wos = nc.dram_tensor("wos", [1024], f32, kind="ExternalInput")
    d_bo8 = nc.dram_tensor("bo8", [1024], f32, kind="ExternalInput")
    d_hm = nc.dram_tensor("hm", [128, 2], f32, kind="ExternalInput")
    d_mn = nc.dram_tensor("mn", [128, 2], f32, kind="ExternalInput")
    d_m2 = nc.dram_tensor("m2", [2, 128], f32, kind="ExternalInput")
    d_m2o = nc.dram_tensor("m2o", [2, 128], f32, kind="ExternalInput")
    d_gb = nc.dram_tensor("gb", [2, 8], f32, kind="ExternalInput")
    d_out = nc.dram_tensor("out", [128, 1024], f32, kind="ExternalOutput")

    cc1_in = nc.dram_tensor("cc1_in", [128, 16], f32)
    cc1_out = nc.dram_tensor("cc1_out", [NC * 128, 16], f32, addr_space="Shared")
    cc2_in = nc.dram_tensor("cc2_in", [128, 1024], f32)
    cc2_out = nc.dram_tensor("cc2_out", [128, 1024], f32, addr_space="Shared")

    def bcast(ap, p=128):
        return bass.AP(tensor=ap.tensor, offset=ap.offset, ap=[[0, p]] + list(ap.ap))

    groups = [list(range(NC))]

    with tile.TileContext(nc) as tc:
        with (
            tc.tile_pool(name="const", bufs=1) as cst_pool,
            tc.tile_pool(name="work", bufs=1) as wk,
            tc.tile_pool(name="psum", bufs=1, space="PSUM") as ps,
            tc.tile_pool(name="psvt", bufs=2, space="PSUM") as psvt,
        ):
            for _rep in range(reps):
                # ---- constant loads (chunked for DMA/compute overlap) ----
                t_xt = [cst_pool.tile([128, 128], fmm, name=f"xt{c}", tag=f"xt{c}") for c in range(8)]
                t_wqk = [cst_pool.tile([128, 512], fmm, name=f"wqk{c}", tag=f"wqk{c}") for c in range(8)]
                t_wv = [cst_pool.tile([128, 256], fmm, name=f"wv{c}", tag=f"wv{c}") for c in range(8)]
                t_wo = [cst_pool.tile([128, 1024], fmm, name=f"wo{k}", tag=f"wo{k}") for k in range(2)]
                t_wos = cst_pool.tile([128, 1024], f32, name="wos", tag="wos")
                t_bo8 = cst_pool.tile([128, 1024], f32, name="bo8", tag="bo8")
                t_hm = cst_pool.tile([128, 2], f32, name="hm", tag="hm")
                t_mn = cst_pool.tile([128, 2], f32, name="mn", tag="mn")
                t_m2 = cst_pool.tile([2, 128], f32, name="m2", tag="m2")
                t_m2o = cst_pool.tile([2, 128], f32, name="m2o", tag="m2o")
                t_gb = cst_pool.tile([2, 8], f32, name="gb", tag="gb")

                for c in range(8):
                    nc.sync.dma_start(out=t_xt[c], in_=d_xt[:, c, :])
                    nc.sync.dma_start(out=t_wqk[c], in_=d_wqk[:, c, :])
                    nc.sync.dma_start(out=t_wv[c], in_=d_wv[:, c, :])
                for k in range(2):
                    nc.sync.dma_start(out=t_wo[k], in_=d_wo[:, k, :])
                nc.sync.dma_start(out=t_hm, in_=d_hm[:])
                nc.sync.dma_start(out=t_mn, in_=d_mn[:])
                nc.sync.dma_start(out=t_m2, in_=d_m2[:])
                nc.sync.dma_start(out=t_m2o, in_=d_m2o[:])
                nc.sync.dma_start(out=t_gb, in_=d_gb[:])
                nc.sync.dma_start(out=t_wos, in_=bcast(d_wos[:]))
                nc.sync.dma_start(out=t_bo8, in_=bcast(d_bo8[:]))

                # ---- phase 1: projections ----
                qk_ps = ps.tile([128, 512], f32, name="qk", tag="qk")
                for c in range(8):
                    nc.tensor.matmul(qk_ps, lhsT=t_xt[c], rhs=t_wqk[c],
                                     start=(c == 0), stop=(c == 7))
                vt_ps = [psvt.tile([128, 128], f32, name="vt", tag="vt") for _ in range(2)]
                for half in range(2):
                    for c in range(8):
                        nc.tensor.matmul(vt_ps[half], lhsT=t_wv[c][:, half * 128:(half + 1) * 128],
                                         rhs=t_xt[c], start=(c == 0), stop=(c == 7))
                vts = wk.tile([128, 2, 128], fmm, name="vts", tag="vts")
                for half in range(2):
                    nc.scalar.copy(out=vts[:, half, :], in_=vt_ps[half])

                # ---- phase 2: payload (per-chunk partial sums) ----
                q_ap = qk_ps[:, 0:256]
                k_ap = qk_ps[:, 256:512]
                ksb = wk.tile([128, 256], f32, name="ksb", tag="ksb")
                nc.scalar.copy(out=ksb, in_=k_ap)
                ksw = wk.tile([128, 256], f32, name="ksw", tag="ksw")
                nc.vector.tensor_copy(out=ksw[0:64, :], in_=ksb[64:128, :])
                nc.vector.tensor_copy(out=ksw[64:128, :], in_=ksb[0:64, :])

                tmp4 = wk.tile([128, 4], f32, name="tmp4", tag="tmp4")
                prod1 = wk.tile([128, 256], f32, name="prod1", tag="prod1")
                prod2 = wk.tile([128, 256], f32, name="prod2", tag="prod2")
                nc.vector.tensor_tensor(out=prod1, in0=q_ap, in1=ksb, op=A.mult)
                nc.vector.tensor_reduce(out=tmp4[:, 0:1], in_=prod1, axis=AX.X, op=A.add)
                nc.vector.tensor_tensor(out=prod2, in0=q_ap, in1=ksw, op=A.mult)
                nc.vector.tensor_reduce(out=tmp4[:, 1:2], in_=prod2, axis=AX.X, op=A.add)
                nc.vector.tensor_reduce(out=tmp4[:, 2:4],
                                        in_=qk_ps[:].rearrange("p (t j) -> p t j", t=2),
                                        axis=AX.X, op=A.add)

                pay = wk.tile([128, 16], f32, name="pay", tag="pay")
                nc.vector.memset(pay, 0.0)
                nc.vector.tensor_scalar(out=pay[:, 0:4], in0=tmp4, scalar1=t_hm[:, 0:1],
                                        scalar2=None, op0=A.mult)
                nc.vector.tensor_scalar(out=pay[:, 4:8], in0=tmp4, scalar1=t_hm[:, 1:2],
                                        scalar2=None, op0=A.mult)
                sq1 = wk.tile([128, 256], f32, name="sq1", tag="sq1")
                sq2 = wk.tile([128, 256], f32, name="sq2", tag="sq2")
                nc.scalar.activation(out=sq1, in_=q_ap, func=AF.Square, accum_out=pay[:, 8:9])
                nc.scalar.activation(out=sq2, in_=k_ap, func=AF.Square, accum_out=pay[:, 9:10])
                vsq = [wk.tile([128, 128], f32, name=f"vsq{i}", tag=f"vsq{i}") for i in range(2)]
                v2ab = [wk.tile([128, 2], f32, name=f"v2ab{i}", tag=f"v2ab{i}") for i in range(2)]
                for half in range(2):
                    nc.scalar.activation(out=vsq[half], in_=vt_ps[half], func=AF.Square)
                    nc.vector.tensor_reduce(out=v2ab[half],
                                            in_=vsq[half][:].rearrange("p (t r) -> p t r", t=2),
                                            axis=AX.X, op=A.add)
                nc.vector.tensor_tensor(out=pay[:, 11:13], in0=v2ab[0], in1=v2ab[1], op=A.add)
                vsab = [wk.tile([128, 2], f32, name=f"vsab{i}", tag=f"vsab{i}") for i in range(2)]
                for half in range(2):
                    nc.vector.tensor_reduce(out=vsab[half],
                                            in_=vt_ps[half][:].rearrange("p (t r) -> p t r", t=2),
                                            axis=AX.X, op=A.add)
                nc.vector.tensor_tensor(out=pay[:, 13:15], in0=vsab[0], in1=vsab[1], op=A.add)

                # ---- collective 1: AllGather payload, local sum ----
                nc.sync.dma_start(out=cc1_in[:], in_=pay)
                if no_cc is True:
                    nc.gpsimd.dma_start(out=cc1_out[0:128, :], in_=cc1_in[:])
                else:
                    nc.gpsimd.collective_compute(
                        "AllGather", A.bypass, replica_groups=groups,
                        ins=[cc1_in[:]], outs=[cc1_out[:]])
                gat = wk.tile([128, 8, 16], f32, name="gat", tag="gat")
                nc.sync.dma_start(out=gat, in_=cc1_out[:].rearrange("(c p) f -> p c f", p=128))
                S = wk.tile([128, 16], f32, name="S", tag="S")
                nc.vector.tensor_reduce(out=S, in_=gat[:].rearrange("p c f -> p f c"),
                                        axis=AX.X, op=A.add)

                # ---- Y matmuls (overlap the AllGather): Y = V @ WoC.T ----
                y_ps = ps.tile([128, 1024], f32, name="y", tag="y")
                for k in range(2):
                    for nn in range(2):
                        nc.tensor.matmul(y_ps[:, nn * 512:(nn + 1) * 512],
                                         lhsT=vts[:, k, :], rhs=t_wo[k][:, nn * 512:(nn + 1) * 512],
                                         start=(k == 0), stop=(k == 1))
                ysw_sb = wk.tile([128, 1024], f32, name="ysw_sb", tag="ysw_sb")
                nc.vector.tensor_copy(out=ysw_sb[0:64, :], in_=y_ps[64:128, :])
                nc.vector.tensor_copy(out=ysw_sb[64:128, :], in_=y_ps[0:64, :])

                # ---- post-gather: head-slot select ----
                S4 = wk.tile([128, 4], f32, name="S4", tag="S4")
                th = wk.tile([128, 4], f32, name="th", tag="th")
                nc.vector.tensor_scalar(out=th, in0=S[:, 0:4], scalar1=t_hm[:, 0:1],
                                        scalar2=None, op0=A.mult)
                nc.vector.scalar_tensor_tensor(out=S4, in0=S[:, 4:8], scalar=t_hm[:, 1:2],
                                               in1=th, op0=A.mult, op1=A.add)
                kr2 = wk.tile([128, 2], f32, name="kr2", tag="kr2")
                nc.vector.tensor_copy(out=kr2[:, 0:1], in_=S4[:, 3:4])
                nc.vector.tensor_copy(out=kr2[0:64, 1:2], in_=S4[64:128, 3:4])
                nc.vector.tensor_copy(out=kr2[64:128, 1:2], in_=S4[0:64, 3:4])

                # ---- global BN1 stats ----
                rhs4 = wk.tile([128, 4], f32, name="rhs4", tag="rhs4")
                nc.vector.tensor_tensor(out=rhs4[:, 0:2], in0=S[:, 2:4], in1=S[:, 6:8], op=A.add)
                nc.vector.tensor_copy(out=rhs4[:, 2:4], in_=S[:, 8:10])
                ones1 = wk.tile([128, 1], f32, name="ones1", tag="ones1")
                nc.vector.memset(ones1, 1.0)
                st_ps = ps.tile([128, 16], f32, name="st", tag="qk")
                nc.tensor.matmul(out=st_ps[0:2, 0:4], lhsT=t_mn, rhs=rhs4, start=True, stop=True)
                nc.tensor.matmul(out=st_ps[0:2, 4:5], lhsT=S[:, 11:13], rhs=ones1,
                                 start=True, stop=True)
                nc.tensor.matmul(out=st_ps[0:2, 5:6], lhsT=S[:, 13:15], rhs=ones1,
                                 start=True, stop=True)
                sts = wk.tile([2, 6], f32, name="sts", tag="sts")   # [Sq Sk Sv SSq SSk SSv]
                nc.vector.tensor_copy(out=sts[:, 0:2], in_=st_ps[0:2, 0:2])
                nc.vector.tensor_copy(out=sts[:, 2:3], in_=st_ps[0:2, 5:6])
                nc.vector.tensor_copy(out=sts[:, 3:5], in_=st_ps[0:2, 2:4])
                nc.vector.tensor_copy(out=sts[:, 5:6], in_=st_ps[0:2, 4:5])

                cst = wk.tile([2, 32], f32, name="cst", tag="cst")
                eps_t = wk.tile([2, 1], f32, name="eps_t", tag="eps_t")
                nc.vector.memset(eps_t, EPS)
                inv_n1 = 1.0 / float(B * INNER)
                nc.vector.tensor_scalar(out=cst[:, 0:3], in0=sts[:, 0:3], scalar1=inv_n1,
                                        scalar2=None, op0=A.mult)          # means
                nc.vector.tensor_scalar(out=cst[:, 3:6], in0=sts[:, 3:6], scalar1=inv_n1,
                                        scalar2=None, op0=A.mult)          # E[x^2]
                nc.vector.tensor_tensor(out=cst[:, 6:9], in0=cst[:, 0:3], in1=cst[:, 0:3], op=A.mult)
                nc.vector.tensor_tensor(out=cst[:, 9:12], in0=cst[:, 3:6], in1=cst[:, 6:9], op=A.subtract)
                nc.scalar.activation(out=cst[:, 12:15], in_=cst[:, 9:12], func=AF.Sqrt,
                                     bias=eps_t, scale=1.0)
                nc.vector.reciprocal(out=cst[:, 15:18], in_=cst[:, 12:15])
                nc.vector.tensor_tensor(out=cst[:, 18:21], in0=t_gb[0:2, 0:3], in1=cst[:, 15:18],
                                        op=A.mult)                          # A = g*rstd
                nc.vector.tensor_tensor(out=cst[:, 24:27], in0=cst[:, 18:21], in1=cst[:, 0:3],
                                        op=A.mult)                          # A*mean
                nc.vector.tensor_tensor(out=cst[:, 21:24], in0=t_gb[0:2, 3:6], in1=cst[:, 24:27],
                                        op=A.subtract)                      # C = b - A*mean

                bc_ps = ps.tile([128, 16], f32, name="bc", tag="vr")
                nc.tensor.matmul(out=bc_ps[:, 0:6], lhsT=t_m2, rhs=cst[:, 18:24],
                                 start=True, stop=True)
                nc.tensor.matmul(out=bc_ps[:, 6:12], lhsT=t_m2o, rhs=cst[:, 18:24],
                                 start=True, stop=True)
                bc = wk.tile([128, 12], f32, name="bc_sb", tag="bc_sb")
                nc.scalar.copy(out=bc, in_=bc_ps[:, 0:12])
                # bc cols: 0 Aq 1 Ak 2 Av 3 Cq 4 Ck 5 Cv | 6 Aq' 7 Ak' 8 Av' 9 Cq' 10 Ck' 11 Cv'

                # ---- scores ----
                CACD = wk.tile([128, 8], f32, name="CACD", tag="CACD")
                in0 = bass.AP(tensor=bc.tensor, offset=bc.offset,
                              ap=[list(bc.ap[0]), [3, 2], [0, 4]])       # [Aq x4, Cq x4]
                in1 = bass.AP(tensor=bc.tensor, offset=bc.offset + 1,
                              ap=[list(bc.ap[0]), [0, 2], [3, 4]])       # [Ak Ck Ak' Ck'] x2
                nc.vector.tensor_tensor(out=CACD[:].rearrange("p (a b) -> p a b", a=2),
                                        in0=in0, in1=in1, op=A.mult)
                nc.vector.tensor_scalar(out=CACD[:, 5:6], in0=CACD[:, 5:6], scalar1=float(DIM),
                                        scalar2=None, op0=A.mult)
                nc.vector.tensor_scalar(out=CACD[:, 7:8], in0=CACD[:, 7:8], scalar1=float(DIM),
                                        scalar2=None, op0=A.mult)

                def cacd(k):
                    return bass.AP(tensor=CACD.tensor, offset=CACD.offset + k,
                                   ap=[list(CACD.ap[0]), [2, 2]])
                CA, CB, CC, CD = cacd(0), cacd(1), cacd(4), cacd(5)

                sc = wk.tile([128, 2], f32, name="sc", tag="sc")
                t3 = wk.tile([128, 2], f32, name="t3", tag="t3")
                nc.vector.tensor_tensor(out=sc, in0=CA, in1=S4[:, 0:2], op=A.mult)
                nc.vector.scalar_tensor_tensor(out=sc, in0=CB, scalar=S4[:, 2:3], in1=sc,
                                               op0=A.mult, op1=A.add)
                nc.vector.tensor_tensor(out=t3, in0=CC, in1=kr2, op=A.mult)
                nc.vector.tensor_tensor(out=sc, in0=sc, in1=t3, op=A.add)
                nc.vector.tensor_tensor(out=sc, in0=sc, in1=CD, op=A.add)

                Dcol = wk.tile([128, 1], f32, name="Dcol", tag="Dcol")
                Din = wk.tile([128, 1], f32, name="Din", tag="Din")
                nc.vector.tensor_reduce(out=Dcol, in_=sc, axis=AX.X, op=A.add)
                nc.vector.reciprocal(out=Din, in_=Dcol)
                w2 = wk.tile([128, 2], f32, name="w2", tag="w2")
                nc.vector.tensor_scalar(out=w2, in0=sc, scalar1=Din, scalar2=None, op0=A.mult)
                uz = wk.tile([128, 3], f32, name="uz", tag="uz")   # [u_diag, u_off, z]
                t4 = wk.tile([128, 1], f32, name="t4", tag="t4")
                nc.vector.tensor_scalar(out=uz[:, 0:1], in0=w2[:, 0:1], scalar1=bc[:, 2:3], scalar2=None, op0=A.mult)
                nc.vector.tensor_scalar(out=uz[:, 1:2], in0=w2[:, 1:2], scalar1=bc[:, 8:9], scalar2=None, op0=A.mult)
                nc.vector.tensor_scalar(out=t4, in0=w2[:, 0:1], scalar1=bc[:, 5:6], scalar2=None, op0=A.mult)
                nc.vector.scalar_tensor_tensor(out=uz[:, 2:3], in0=w2[:, 1:2], scalar=bc[:, 11:12],
                                               in1=t4, op0=A.mult, op1=A.add)

                # ---- combine: partial = u_diag*Y + u_off*Ysw + z*wo_sum + bo/8 ----
                Rt = wk.tile([128, 1024], f32, name="Rt", tag="Rt")
                nc.vector.scalar_tensor_tensor(out=Rt, in0=t_wos, scalar=uz[:, 2:3], in1=t_bo8,
                                               op0=A.mult, op1=A.add)
                t2 = wk.tile([128, 1024], f32, name="t2", tag="t2")
                nc.vector.scalar_tensor_tensor(out=t2, in0=ysw_sb, scalar=uz[:, 1:2], in1=Rt,
                                               op0=A.mult, op1=A.add)
                outp = wk.tile([128, 1024], f32, name="outp", tag="outp")
                nc.vector.scalar_tensor_tensor(out=outp, in0=y_ps, scalar=uz[:, 0:1], in1=t2,
                                               op0=A.mult, op1=A.add)

                # ---- collective 2: AllReduce partials ----
                nc.sync.dma_start(out=cc2_in[:], in_=outp)
                if no_cc in (True, "no_ar"):
                    nc.gpsimd.dma_start(out=cc2_out[:], in_=cc2_in[:])
                else:
                    nc.gpsimd.collective_compute(
                        "AllReduce", A.add, replica_groups=groups,
                        ins=[cc2_in[:]], outs=[cc2_out[:]])
                Xt = wk.tile([128, 1024], f32, name="Xt", tag="Xt")
                nc.sync.dma_start(out=Xt[:, 0:512], in_=cc2_out[:, 0:512])
                nc.sync.dma_start(out=Xt[:, 512:1024], in_=cc2_out[:, 512:1024])

                # ---- BN2 (halves overlap the Xt DMA) ----
                r2h = wk.tile([128, 2, 2], f32, name="r2h", tag="r2h")
                scr2 = wk.tile([128, 1024], f32, name="scr2", tag="scr2")
                for hh in range(2):
                    cols = slice(hh * 512, (hh + 1) * 512)
                    nc.vector.tensor_reduce(out=r2h[:, hh, 0:1], in_=Xt[:, cols], axis=AX.X, op=A.add)
                    nc.scalar.activation(out=scr2[:, cols], in_=Xt[:, cols], func=AF.Square,
                                         accum_out=r2h[:, hh, 1:2])
                r2 = wk.tile([128, 2], f32, name="r2", tag="r2")
                nc.vector.tensor_tensor(out=r2, in0=r2h[:, 0, :], in1=r2h[:, 1, :], op=A.add)
                st2_ps = ps.tile([128, 4], f32, name="st2", tag="qk")
                nc.tensor.matmul(out=st2_ps[0:2, 0:2], lhsT=t_mn, rhs=r2, start=True, stop=True)
                cst2 = wk.tile([2, 12], f32, name="cst2", tag="cst2")
                inv_n2 = 1.0 / float(B * DIM)
                nc.vector.tensor_scalar(out=cst2[:, 0:2], in0=st2_ps[0:2, 0:2], scalar1=inv_n2,
                                        scalar2=None, op0=A.mult)           # [mean, E2]
                nc.vector.tensor_tensor(out=cst2[:, 2:3], in0=cst2[:, 0:1], in1=cst2[:, 0:1], op=A.mult)
                nc.vector.tensor_tensor(out=cst2[:, 3:4], in0=cst2[:, 1:2], in1=cst2[:, 2:3], op=A.subtract)
                nc.scalar.activation(out=cst2[:, 4:5], in_=cst2[:, 3:4], func=AF.Sqrt,
                                     bias=eps_t, scale=1.0)
                nc.vector.reciprocal(out=cst2[:, 5:6], in_=cst2[:, 4:5])
                nc.vector.tensor_tensor(out=cst2[:, 6:7], in0=t_gb[0:2, 6:7], in1=cst2[:, 5:6], op=A.mult)  # abn
                nc.vector.tensor_tensor(out=cst2[:, 8:9], in0=cst2[:, 6:7], in1=cst2[:, 0:1], op=A.mult)
                nc.vector.tensor_tensor(out=cst2[:, 7:8], in0=t_gb[0:2, 7:8], in1=cst2[:, 8:9], op=A.subtract)  # cbn
                bc2_ps = ps.tile([128, 4], f32, name="bc2", tag="vr")
                nc.tensor.matmul(out=bc2_ps[:, 0:2], lhsT=t_m2, rhs=cst2[:, 6:8], start=True, stop=True)
                bc2 = wk.tile([128, 2], f32, name="bc2_sb", tag="bc2_sb")
                nc.scalar.copy(out=bc2, in_=bc2_ps[:, 0:2])
                fin = wk.tile([128, 1024], f32, name="fin", tag="fin")
                for hh in range(2):
                    cols = slice(hh * 512, (hh + 1) * 512)
                    nc.vector.tensor_scalar(out=fin[:, cols], in0=Xt[:, cols], scalar1=bc2[:, 0:1],
                                            scalar2=bc2[:, 1:2], op0=A.mult, op1=A.add)
                    nc.sync.dma_start(out=d_out[:, cols], in_=fin[:, cols])

    nc.compile()
    return nc


def _prep_inputs_v1(x, Wq, Wk, Wv, Wo, bo, g_q, b_q, g_k, b_k, g_v, b_v, g_bn, b_bn):
    f = np.float32
    x, Wq, Wk, Wv, Wo, bo = (np.asarray(t, f) for t in (x, Wq, Wk, Wv, Wo, bo))
    g_q, b_q, g_k, b_k, g_v, b_v, g_bn, b_bn = (
        np.asarray(t, f) for t in (g_q, b_q, g_k, b_k, g_v, b_v, g_bn, b_bn))
    x = np.ascontiguousarray(x, f)
    xf = x.reshape(B, N, DIM)
    Xr = np.ascontiguousarray(xf.transpose(1, 0, 2).reshape(N * B, DIM))   # n-major rows
    xt = np.ascontiguousarray(Xr.T.reshape(8, 128, 128).transpose(1, 0, 2))  # [p, c, r]

    mn = np.zeros((128, 2), f)
    mn[0:64, 0] = 1.0
    mn[64:128, 1] = 1.0
    m2 = np.ascontiguousarray(mn.T)            # (2, 128)
    m2o = np.ascontiguousarray(mn[:, ::-1].T)  # opposite channel
    gb = np.stack([g_q, g_k, g_v, b_q, b_k, b_v, g_bn, b_bn], axis=1).astype(f)
    bo8 = (np.asarray(bo, f) / NC).astype(f)

    in_maps = []
    for i in range(NC):
        rows = slice(i * DPC, (i + 1) * DPC)
        head = i // 4
        wqk_c = np.concatenate([Wq[rows], Wk[rows]], axis=0).astype(f)       # (512, 1024)
        wqk = np.ascontiguousarray(wqk_c.T.reshape(8, 128, 512).transpose(1, 0, 2))
        wv_c = np.asarray(Wv[rows], f)                                        # (256, 1024)
        wv = np.ascontiguousarray(wv_c.T.reshape(8, 128, 256).transpose(1, 0, 2))
        WoC = np.asarray(Wo[:, rows], f)                                      # (1024, 256)
        wo = np.ascontiguousarray(WoC.T.reshape(2, 128, 1024).transpose(1, 0, 2))
        wos = np.ascontiguousarray(WoC.sum(1))                                # (1024,)
        hm = np.zeros((128, 2), f)
        hm[:, head] = 1.0
        in_maps.append({
            "xt": xt, "wqk": wqk, "wv": wv, "wo": wo,
            "wos": wos, "bo8": bo8, "hm": hm, "mn": mn, "m2": m2,
            "m2o": m2o, "gb": gb,
        })
    return in_maps


# --------------------------------------------------------------------------
# driver
# --------------------------------------------------------------------------

def _prep_inputs_v7(x, Wq, Wk, Wv, Wo, bo, g_q, b_q, g_k, b_k, g_v, b_v, g_bn, b_bn):
    """v2 prep + int16 fixed-point scales computed from the exact inputs."""
    f = np.float32
    in_maps = _prep_inputs_v2(x, Wq, Wk, Wv, Wo, bo, g_q, b_q, g_k, b_k,
                              g_v, b_v, g_bn, b_bn)
    xf = np.asarray(x, f).reshape(B, N, DIM)
    Xr = np.ascontiguousarray(xf.transpose(1, 0, 2).reshape(N * B, DIM))
    q = Xr @ np.asarray(Wq, f).T        # (128, 2048)
    k = Xr @ np.asarray(Wk, f).T
    v = Xr @ np.asarray(Wv, f).T
    ksw_rows = np.concatenate([np.arange(64, 128), np.arange(0, 64)])

    M = np.ones(16, f)                   # per-payload-column absmax over cores
    ysum_abs = np.zeros((128, DIM), f)
    Wof = np.asarray(Wo, f)
    for c in range(NC):
        sl = slice(c * DPC, (c + 1) * DPC)
        qc, kc, vc = q[:, sl], k[:, sl], v[:, sl]
        slot = 0 if c < 4 else 4
        M[slot + 0] = max(M[slot + 0], np.abs((qc * kc).sum(1)).max())
        M[slot + 1] = max(M[slot + 1], np.abs((qc * kc[ksw_rows]).sum(1)).max())
        M[slot + 2] = max(M[slot + 2], np.abs(qc.sum(1)).max())
        M[slot + 3] = max(M[slot + 3], np.abs(kc.sum(1)).max())
        M[8] = max(M[8], (qc ** 2).sum(1).max())
        M[9] = max(M[9], (kc ** 2).sum(1).max())
        vsq = vc ** 2
        ch_sq = np.stack([vsq[0:64].sum(0), vsq[64:128].sum(0)])     # (2, 256)
        ch_s = np.stack([vc[0:64].sum(0), vc[64:128].sum(0)])
        M[11] = max(M[11], np.abs(ch_sq[:, 0:128] + ch_sq[:, 128:256]).max())
        M[12] = M[11]
        M[13] = max(M[13], np.abs(ch_s[:, 0:128] + ch_s[:, 128:256]).max())
        M[14] = M[13]
        ysum_abs += np.abs(vc @ Wof[:, sl].T)
    sY = f(30000.0 / max(ysum_abs.max(), 1e-3))
    psc_row = (2.0 ** 22) / (M * 2.0)
    psc = np.broadcast_to(psc_row.astype(f), (128, 16)).copy()
    ipsc = np.broadcast_to((1.0 / psc_row).astype(f), (128, 16)).copy()
    isy = np.full((128, 1), 1.0 / sY, f)
    for i in range(NC):
        in_maps[i]["psc"] = psc
        in_maps[i]["ipsc"] = ipsc
        in_maps[i]["isy"] = isy
        in_maps[i]["hmy"] = (in_maps[i]["hm"] * sY).astype(f)
    return in_maps


def _prep_inputs(**inputs):
    if STRATEGY == "v1":
        return _prep_inputs_v1(**inputs)
    if STRATEGY in ("v7", "v8", "v9"):
        return _prep_inputs_v7(**inputs)
    return _prep_inputs_v2(**inputs)


def _postprocess(out128):
    return np.ascontiguousarray(
        out128.reshape(N, B, DIM).transpose(1, 0, 2).reshape(B, N, H, W)
    ).astype(np.float32)


def _get_program(reps=1):
    key = ("nc", STRATEGY, MM_DT, reps, NO_CC, CC_F32, CC_DIM, CC_UT)
    if key not in _PROG_CACHE:
        _PROG_CACHE[key] = _build_program(MM_DT, reps)
    return _PROG_CACHE[key]


def kernel(**inputs):
    from concourse.bass_utils import run_bass_kernel_spmd
    nc = _get_program()
    in_maps = _prep_inputs(**inputs)
    res = run_bass_kernel_spmd(nc, in_maps, list(range(NC)))
    return _postprocess(res.results[0]["out"])


def run_traced(inputs):
    """Like kernel() but with NTFF tracing; returns (output, BassKernelResults)."""
    from concourse.bass_utils import run_bass_kernel_spmd
    nc = _get_program()
    in_maps = _prep_inputs(**inputs)
    res = run_bass_kernel_spmd(nc, in_maps, list(range(NC)), trace=True)
    return _postprocess(res.results[0]["out"]), res


def run_sim(inputs):
    """Validate in the multi-core simulator; returns output."""
    from concourse.bass_interp import MultiCoreSim
    nc = _get_program()
    in_maps = _prep_inputs(**inputs)
    sim = MultiCoreSim(nc, num_cores=NC, trace=False)
    for i in range(NC):
        for k, v in in_maps[i].items():
            sim.cores[i].tensor(k)[:] = v
    sim.simulate()
    return _postprocess(np.array(sim.cores[0].tensor("out")))


# revision 30
# speedup vs baseline: 1.4007x; 1.4007x over previous
"""Trainium2 Bass kernel for nn_Attention_46067819217077 (sparse_attention).

Computation (reference):
  x (64,2,32,32) -> flatten (b=64, n=2, dim=1024)
  q/k/v = BN1d_n( x @ W{q,k,v}.T )          (inner = 2048 = 2 heads x 1024)
  linear attention per (b, head):  out = (s @ v_hat) * D_inv  with
      s[n,m] = q_hat[n] . k_hat[m],  D[n] = s[n,0]+s[n,1]   (seq len n = 2)
  out = merge_heads @ Wo.T + bo ; BN2d over (b, H, W)

Strategy: 8-way tensor-parallel on the inner (head-feature) axis.
Each core owns a 256-wide chunk of Wq/Wk/Wv rows and of Wo columns.
BatchNorm statistics, q.k dot products and row sums are all *linear* in
per-chunk partial sums, so small collectives provide everything needed.

v1 (legacy): AllGather tiny payload -> local sum -> compute attention
  scalars -> combine into per-core partial -> AllReduce [128,1024] f32.
  The two collectives are serially dependent.

v2: two *independent* collectives. CC-A: AllReduce the [128,16] f32
  payload. CC-B: AllReduce raw Y = V @ WoC.T, head-packed into
  [128,2048] fp16 (head-0 cores fill cols 0:1024, head-1 cores cols
  1024:2048; zeros elsewhere). All attention/BN scalar math runs while
  CC-B is in flight; the final combine applies per-row scalars to the
  two head sums:
    X = ud0*Y0 + uo0*Y0_sw + ud1*Y1 + uo1*Y1_sw + z0*wos0 + z1*wos1 + bo
  BN2 is computed redundantly on every core. V/Wo matmuls run in fp16
  (the q/k path must stay fp32: 1/D amplifies q/k rounding ~20x).

v2m: like v2 but one merged f32 AllReduce [128, 2064] (Y head-packed
  f32 + payload in the last 16 cols).

v3/v3m: v2/v2m + cross-rep pipelining (bufs=2 pools, parity-double-
  buffered collective staging tensors).

v6: v3 + software-pipelined emission: phase1(k+1) is emitted before
  phase2(k) so no engine queue blocks the next rep's collective inputs
  behind the current rep's post-collective tail.

v7 (default, fastest): v6 with ONE int16 AllReduce [128, 2080] per rep.
  Measured: each collective ring op costs ~9us latency regardless of
  size and consecutive collectives never overlap, so collective COUNT
  dominates. Y rides as int16 (scale sY derived from the actual inputs
  host-side, with 2x margins); the payload needs f32-grade precision
  (1/D amplifies errors ~20x) which fp16 wire-adds cannot give, but
  integer wire-adds are EXACT: each f32 payload value v is carried as
  hi = i16(v*s/2048), lo = i16(v*s - 2048*hi) and decoded as
  (sum(hi)*2048 + sum(lo))/s after the reduce. HW rel err 4.4e-4.

Row layout everywhere: r = n*64 + b  (channel-major, 128 rows).
"""

import numpy as np

NC = 8
B, N, H, W = 64, 2, 32, 32
DIM = H * W                # 1024
INNER = DIM * 2            # 2048
DPC = INNER // NC          # 256 per-core chunk
EPS = 1e-5

_PROG_CACHE = {}

# Matmul operand dtype for the q/k path: "f32" (exact, 4 cycles/row) or
# "f32r" (fast fp32, reduced precision). V/Wo path in v2 is always fp16.
MM_DT = "f32r"    # QK-path matmul dtype; f32r measured 5.9e-3 rel err on HW
STRATEGY = "v9"   # v9 = v8 + head-pack on act engine (see _build_v6 act_pack)
NO_CC = False     # False | True | "no_a" | "no_b"  (local-DMA substitutes)
CC_F32 = False    # CC-B (Y AllReduce) in f32 instead of fp16
CC_DIM = "Partition"  # cc_dim for the packed AllReduce (Free vs Partition: no consistent diff)
CC_UT = "No"      # unique_tensors hint for the packed AllReduce


def _build_program(mm_dt=None, reps=1, strategy=None):
    strategy = strategy or STRATEGY
    if strategy == "v1":
        return _build_v1(mm_dt, reps)
    if strategy == "v6":
        return _build_v6(mm_dt, reps)
    if strategy == "v7":
        return _build_v6(mm_dt, reps, packed=True)
    if strategy == "v8":
        return _build_v6(mm_dt, reps, packed=True, slim=True)
    if strategy == "v9":
        return _build_v6(mm_dt, reps, packed=True, slim=True, act_pack=True)
    return _build_v2(mm_dt, reps, merged=(strategy in ("v2m", "v3m")),
                     pipelined=(strategy in ("v3", "v3m")))


# --------------------------------------------------------------------------
# v2: independent collectives, fp16 V/Wo path
# --------------------------------------------------------------------------

def _build_v2(mm_dt=None, reps=1, merged=False, pipelined=False):
    import concourse.bass as bass
    import concourse.mybir as mybir
    import concourse.tile as tile
    from concourse import bacc

    f32 = mybir.dt.float32
    f16 = mybir.dt.float16
    fmm = mybir.dt.float32r if (mm_dt or MM_DT) == "f32r" else mybir.dt.float32
    fcc = f32 if (merged or CC_F32) else f16   # dtype of the big Y collective
    A = mybir.AluOpType
    AF = mybir.ActivationFunctionType
    AX = mybir.AxisListType

    no_cc = NO_CC
    nc = bacc.Bacc(None, target_bir_lowering=False, debug=False, num_devices=NC)

    # ---- I/O ----
    d_xt = nc.dram_tensor("xt", [128, 8, 128], fmm, kind="ExternalInput")
    d_wqk = nc.dram_tensor("wqk", [128, 8, 512], fmm, kind="ExternalInput")
    d_wv = nc.dram_tensor("wv", [128, 8, 256], f16, kind="ExternalInput")
    d_wo = nc.dram_tensor("wo", [128, 2, 1024], f16, kind="ExternalInput")
    d_wos = nc.dram_tensor("wos", [2, 1024], f32, kind="ExternalInput")
    d_bo = nc.dram_tensor("bo", [1024], f32, kind="ExternalInput")
    d_hm = nc.dram_tensor("hm", [128, 2], f32, kind="ExternalInput")
    d_mn = nc.dram_tensor("mn", [128, 2], f32, kind="ExternalInput")
    d_m2 = nc.dram_tensor("m2", [2, 128], f32, kind="ExternalInput")
    d_gb = nc.dram_tensor("gb", [2, 8], f32, kind="ExternalInput")
    d_out = nc.dram_tensor("out", [128, 1024], f32, kind="ExternalOutput")

    ncc = 2 if pipelined else 1   # double-buffer collective staging by rep parity
    if merged:
        ccm_in = [nc.dram_tensor(f"ccm_in{i}", [128, 2064], f32) for i in range(ncc)]
        ccm_out = [nc.dram_tensor(f"ccm_out{i}", [128, 2064], f32, addr_space="Shared")
                   for i in range(ncc)]
    else:
        cc1_in = [nc.dram_tensor(f"cc1_in{i}", [128, 16], f32) for i in range(ncc)]
        cc1_out = [nc.dram_tensor(f"cc1_out{i}", [128, 16], f32, addr_space="Shared")
                   for i in range(ncc)]
        cc2_in = [nc.dram_tensor(f"cc2_in{i}", [128, 2048], fcc) for i in range(ncc)]
        cc2_out = [nc.dram_tensor(f"cc2_out{i}", [128, 2048], fcc, addr_space="Shared")
                   for i in range(ncc)]

    def bcast(ap, p=128):
        return bass.AP(tensor=ap.tensor, offset=ap.offset, ap=[[0, p]] + list(ap.ap))

    groups = [list(range(NC))]

    cb = 2 if pipelined else 1
    with tile.TileContext(nc) as tc:
        with (
            tc.tile_pool(name="const", bufs=cb) as cst_pool,
            tc.tile_pool(name="work", bufs=1) as wk,
            tc.tile_pool(name="psum", bufs=1, space="PSUM") as ps,
            tc.tile_pool(name="psvt", bufs=2, space="PSUM") as psvt,
        ):
            for _rep in range(reps):
                pp = _rep % ncc
                # ---- constant loads (chunked for DMA/compute overlap) ----
                t_xt = [cst_pool.tile([128, 128], fmm, name=f"xt{c}", tag=f"xt{c}") for c in range(8)]
                t_wqk = [cst_pool.tile([128, 512], fmm, name=f"wqk{c}", tag=f"wqk{c}") for c in range(8)]
                t_wv = [cst_pool.tile([128, 256], f16, name=f"wv{c}", tag=f"wv{c}") for c in range(8)]
                t_wo = [cst_pool.tile([128, 1024], f16, name=f"wo{k}", tag=f"wo{k}") for k in range(2)]
                t_wos = cst_pool.tile([128, 2, 1024], f32, name="wos", tag="wos")
                t_bo = cst_pool.tile([128, 1024], f32, name="bo", tag="bo")
                t_hm = cst_pool.tile([128, 2], f32, name="hm", tag="hm")
                t_mn = cst_pool.tile([128, 2], f32, name="mn", tag="mn")
                t_m2 = cst_pool.tile([2, 128], f32, name="m2", tag="m2")
                t_gb = cst_pool.tile([2, 8], f32, name="gb", tag="gb")

                for c in range(8):
                    nc.sync.dma_start(out=t_xt[c], in_=d_xt[:, c, :])
                    nc.sync.dma_start(out=t_wqk[c], in_=d_wqk[:, c, :])
                for c in range(8):
                    nc.sync.dma_start(out=t_wv[c], in_=d_wv[:, c, :])
                for k in range(2):
                    nc.sync.dma_start(out=t_wo[k], in_=d_wo[:, k, :])
                nc.sync.dma_start(out=t_hm, in_=d_hm[:])
                nc.sync.dma_start(out=t_mn, in_=d_mn[:])
                nc.sync.dma_start(out=t_m2, in_=d_m2[:])
                nc.sync.dma_start(out=t_gb, in_=d_gb[:])
                nc.sync.dma_start(out=t_wos, in_=bcast(d_wos[:]))
                nc.sync.dma_start(out=t_bo, in_=bcast(d_bo[:]))

                # fp16 cast of x chunks for the V path
                t_x16 = [wk.tile([128, 128], f16, name=f"x16_{c}", tag=f"x16_{c}", bufs=cb)
                         for c in range(8)]
                for c in range(8):
                    nc.scalar.copy(out=t_x16[c], in_=t_xt[c])

                # ---- phase 1: projections ----
                qk_ps = ps.tile([128, 512], f32, name="qk", tag="qk", bufs=cb)
                for c in range(8):
                    nc.tensor.matmul(qk_ps, lhsT=t_xt[c], rhs=t_wqk[c],
                                     start=(c == 0), stop=(c == 7))
                # V^T tiles: out[j, r] += wv_c.T @ x16_c   (fp16)
                vt_ps = [psvt.tile([128, 128], f32, name="vt", tag="vt") for _ in range(2)]
                for half in range(2):
                    for c in range(8):
                        nc.tensor.matmul(vt_ps[half], lhsT=t_wv[c][:, half * 128:(half + 1) * 128],
                                         rhs=t_x16[c], start=(c == 0), stop=(c == 7))

                # ---- phase 2: payload (per-chunk partial sums) ----
                q_ap = qk_ps[:, 0:256]
                k_ap = qk_ps[:, 256:512]
                ksb = wk.tile([128, 256], f32, name="ksb", tag="ksb")
                nc.scalar.copy(out=ksb, in_=k_ap)
                ksw = wk.tile([128, 256], f32, name="ksw", tag="ksw")
                nc.vector.tensor_copy(out=ksw[0:64, :], in_=ksb[64:128, :])
                nc.vector.tensor_copy(out=ksw[64:128, :], in_=ksb[0:64, :])

                tmp4 = wk.tile([128, 4], f32, name="tmp4", tag="tmp4")
                prod1 = wk.tile([128, 256], f32, name="prod1", tag="prod1")
                prod2 = wk.tile([128, 256], f32, name="prod2", tag="prod2")
                nc.vector.tensor_tensor(out=prod1, in0=q_ap, in1=ksb, op=A.mult)
                nc.vector.tensor_reduce(out=tmp4[:, 0:1], in_=prod1, axis=AX.X, op=A.add)
                nc.vector.tensor_tensor(out=prod2, in0=q_ap, in1=ksw, op=A.mult)
                nc.vector.tensor_reduce(out=tmp4[:, 1:2], in_=prod2, axis=AX.X, op=A.add)
                nc.vector.tensor_reduce(out=tmp4[:, 2:4],
                                        in_=qk_ps[:].rearrange("p (t j) -> p t j", t=2),
                                        axis=AX.X, op=A.add)

                pay = wk.tile([128, 16], f32, name="pay", tag="pay", bufs=cb)
                nc.vector.memset(pay, 0.0)
                nc.vector.tensor_scalar(out=pay[:, 0:4], in0=tmp4, scalar1=t_hm[:, 0:1],
                                        scalar2=None, op0=A.mult)
                nc.vector.tensor_scalar(out=pay[:, 4:8], in0=tmp4, scalar1=t_hm[:, 1:2],
                                        scalar2=None, op0=A.mult)
                sq1 = wk.tile([128, 256], f32, name="sq1", tag="sq1")
                sq2 = wk.tile([128, 256], f32, name="sq2", tag="sq2")
                nc.scalar.activation(out=sq1, in_=q_ap, func=AF.Square, accum_out=pay[:, 8:9])
                nc.scalar.activation(out=sq2, in_=k_ap, func=AF.Square, accum_out=pay[:, 9:10])
                vsq = [wk.tile([128, 128], f32, name=f"vsq{i}", tag=f"vsq{i}") for i in range(2)]
                v2ab = [wk.tile([128, 2], f32, name=f"v2ab{i}", tag=f"v2ab{i}") for i in range(2)]
                for half in range(2):
                    nc.scalar.activation(out=vsq[half], in_=vt_ps[half], func=AF.Square)
                    nc.vector.tensor_reduce(out=v2ab[half],
                                            in_=vsq[half][:].rearrange("p (t r) -> p t r", t=2),
                                            axis=AX.X, op=A.add)
                nc.vector.tensor_tensor(out=pay[:, 11:13], in0=v2ab[0], in1=v2ab[1], op=A.add)
                vsab = [wk.tile([128, 2], f32, name=f"vsab{i}", tag=f"vsab{i}") for i in range(2)]
                for half in range(2):
                    nc.vector.tensor_reduce(out=vsab[half],
                                            in_=vt_ps[half][:].rearrange("p (t r) -> p t r", t=2),
                                            axis=AX.X, op=A.add)
                nc.vector.tensor_tensor(out=pay[:, 13:15], in0=vsab[0], in1=vsab[1], op=A.add)

                # ---- Y matmuls: Y = V @ WoC.T  (fp16) ----
                vts = wk.tile([128, 2, 128], f16, name="vts", tag="vts", bufs=cb)
                for half in range(2):
                    nc.scalar.copy(out=vts[:, half, :], in_=vt_ps[half])
                y_ps = ps.tile([128, 1024], f32, name="y", tag="y")
                for k in range(2):
                    for nn in range(2):
                        nc.tensor.matmul(y_ps[:, nn * 512:(nn + 1) * 512],
                                         lhsT=vts[:, k, :], rhs=t_wo[k][:, nn * 512:(nn + 1) * 512],
                                         start=(k == 0), stop=(k == 1))
                # head-pack Y into [128, 2048] via the per-core head mask
                yh = wk.tile([128, 2048], fcc, name="yh", tag="yh", bufs=cb)
                nc.vector.tensor_scalar(out=yh[:, 0:1024], in0=y_ps, scalar1=t_hm[:, 0:1],
                                        scalar2=None, op0=A.mult)
                nc.vector.tensor_scalar(out=yh[:, 1024:2048], in0=y_ps, scalar1=t_hm[:, 1:2],
                                        scalar2=None, op0=A.mult)

                # ---- collectives ----
                if merged:
                    nc.sync.dma_start(out=ccm_in[pp][:, 0:2048], in_=yh)
                    nc.sync.dma_start(out=ccm_in[pp][:, 2048:2064], in_=pay)
                    if no_cc:
                        nc.gpsimd.dma_start(out=ccm_out[pp][:], in_=ccm_in[pp][:])
                    else:
                        nc.gpsimd.collective_compute(
                            "AllReduce", A.add, replica_groups=groups,
                            ins=[ccm_in[pp][:]], outs=[ccm_out[pp][:]])
                    S = wk.tile([128, 16], f32, name="S", tag="S")
                    nc.sync.dma_start(out=S, in_=ccm_out[pp][:, 2048:2064])
                else:
                    # CC-A: tiny payload AllReduce (issued first, small)
                    nc.sync.dma_start(out=cc1_in[pp][:], in_=pay)
                    if no_cc in (True, "no_a"):
                        nc.gpsimd.dma_start(out=cc1_out[pp][:], in_=cc1_in[pp][:])
                    else:
                        nc.gpsimd.collective_compute(
                            "AllReduce", A.add, replica_groups=groups,
                            ins=[cc1_in[pp][:]], outs=[cc1_out[pp][:]])
                    # CC-B: head-packed Y AllReduce (independent of CC-A)
                    nc.sync.dma_start(out=cc2_in[pp][:], in_=yh)
                    if no_cc in (True, "no_b"):
                        nc.gpsimd.dma_start(out=cc2_out[pp][:], in_=cc2_in[pp][:])
                    else:
                        nc.gpsimd.collective_compute(
                            "AllReduce", A.add, replica_groups=groups,
                            ins=[cc2_in[pp][:]], outs=[cc2_out[pp][:]])
                    S = wk.tile([128, 16], f32, name="S", tag="S")
                    nc.sync.dma_start(out=S, in_=cc1_out[pp][:])

                # ---- global BN1 stats (overlaps CC-B flight) ----
                rhs4 = wk.tile([128, 4], f32, name="rhs4", tag="rhs4")
                nc.vector.tensor_tensor(out=rhs4[:, 0:2], in0=S[:, 2:4], in1=S[:, 6:8], op=A.add)
                nc.vector.tensor_copy(out=rhs4[:, 2:4], in_=S[:, 8:10])
                ones1 = wk.tile([128, 1], f32, name="ones1", tag="ones1")
                nc.vector.memset(ones1, 1.0)
                st_ps = ps.tile([128, 16], f32, name="st", tag="sm")
                nc.tensor.matmul(out=st_ps[0:2, 0:4], lhsT=t_mn, rhs=rhs4, start=True, stop=True)
                nc.tensor.matmul(out=st_ps[0:2, 4:5], lhsT=S[:, 11:13], rhs=ones1,
                                 start=True, stop=True)
                nc.tensor.matmul(out=st_ps[0:2, 5:6], lhsT=S[:, 13:15], rhs=ones1,
                                 start=True, stop=True)
                sts = wk.tile([2, 6], f32, name="sts", tag="sts")   # [Sq Sk Sv SSq SSk SSv]
                nc.vector.tensor_copy(out=sts[:, 0:2], in_=st_ps[0:2, 0:2])
                nc.vector.tensor_copy(out=sts[:, 2:3], in_=st_ps[0:2, 5:6])
                nc.vector.tensor_copy(out=sts[:, 3:5], in_=st_ps[0:2, 2:4])
                nc.vector.tensor_copy(out=sts[:, 5:6], in_=st_ps[0:2, 4:5])

                cst = wk.tile([2, 32], f32, name="cst", tag="cst")
                eps_t = wk.tile([2, 1], f32, name="eps_t", tag="eps_t")
                nc.vector.memset(eps_t, EPS)
                inv_n1 = 1.0 / float(B * INNER)
                nc.vector.tensor_scalar(out=cst[:, 0:3], in0=sts[:, 0:3], scalar1=inv_n1,
                                        scalar2=None, op0=A.mult)          # means
                nc.vector.tensor_scalar(out=cst[:, 3:6], in0=sts[:, 3:6], scalar1=inv_n1,
                                        scalar2=None, op0=A.mult)          # E[x^2]
                nc.vector.tensor_tensor(out=cst[:, 6:9], in0=cst[:, 0:3], in1=cst[:, 0:3], op=A.mult)
                nc.vector.tensor_tensor(out=cst[:, 9:12], in0=cst[:, 3:6], in1=cst[:, 6:9], op=A.subtract)
                nc.scalar.activation(out=cst[:, 12:15], in_=cst[:, 9:12], func=AF.Sqrt,
                                     bias=eps_t, scale=1.0)
                nc.vector.reciprocal(out=cst[:, 15:18], in_=cst[:, 12:15])
                nc.vector.tensor_tensor(out=cst[:, 18:21], in0=t_gb[0:2, 0:3], in1=cst[:, 15:18],
                                        op=A.mult)                          # A = g*rstd
                nc.vector.tensor_tensor(out=cst[:, 24:27], in0=cst[:, 18:21], in1=cst[:, 0:3],
                                        op=A.mult)                          # A*mean
                nc.vector.tensor_tensor(out=cst[:, 21:24], in0=t_gb[0:2, 3:6], in1=cst[:, 24:27],
                                        op=A.subtract)                      # C = b - A*mean

                # broadcast per-channel constants to rows: same + opposite channel
                bc_ps = ps.tile([128, 16], f32, name="bc", tag="vr")
                nc.tensor.matmul(out=bc_ps[:, 0:6], lhsT=t_m2, rhs=cst[:, 18:24],
                                 start=True, stop=True)
                bc = wk.tile([128, 12], f32, name="bc_sb", tag="bc_sb")
                nc.scalar.copy(out=bc[:, 0:6], in_=bc_ps[:, 0:6])
                # opposite-channel constants: swap row halves of bc[:,0:6]
                nc.vector.tensor_copy(out=bc[0:64, 6:12], in_=bc[64:128, 0:6])
                nc.vector.tensor_copy(out=bc[64:128, 6:12], in_=bc[0:64, 0:6])
                # bc cols: 0 Aq 1 Ak 2 Av 3 Cq 4 Ck 5 Cv | 6 Aq' 7 Ak' 8 Av' 9 Cq' 10 Ck' 11 Cv'

                # ---- scores: CACD coefficients (channel-based, head-independent) ----
                CACD = wk.tile([128, 8], f32, name="CACD", tag="CACD")
                in0 = bass.AP(tensor=bc.tensor, offset=bc.offset,
                              ap=[list(bc.ap[0]), [3, 2], [0, 4]])       # [Aq x4, Cq x4]
                in1 = bass.AP(tensor=bc.tensor, offset=bc.offset + 1,
                              ap=[list(bc.ap[0]), [0, 2], [3, 4]])       # [Ak Ck Ak' Ck'] x2
                nc.vector.tensor_tensor(out=CACD[:].rearrange("p (a b) -> p a b", a=2),
                                        in0=in0, in1=in1, op=A.mult)
                nc.vector.tensor_scalar(out=CACD[:, 5:6], in0=CACD[:, 5:6], scalar1=float(DIM),
                                        scalar2=None, op0=A.mult)
                nc.vector.tensor_scalar(out=CACD[:, 7:8], in0=CACD[:, 7:8], scalar1=float(DIM),
                                        scalar2=None, op0=A.mult)

                def cacd(k):
                    return bass.AP(tensor=CACD.tensor, offset=CACD.offset + k,
                                   ap=[list(CACD.ap[0]), [2, 2]])
                CA, CB, CC, CD = cacd(0), cacd(1), cacd(4), cacd(5)

                # per-head attention scalars -> uz6 [ud0 uo0 z0 ud1 uo1 z1]
                uz6 = wk.tile([128, 6], f32, name="uz6", tag="uz6")
                for h in range(2):
                    S4h = S[:, 4 * h:4 * h + 4]
                    kr2 = wk.tile([128, 2], f32, name=f"kr2_{h}", tag=f"kr2_{h}")
                    nc.vector.tensor_copy(out=kr2[:, 0:1], in_=S4h[:, 3:4])
                    nc.vector.tensor_copy(out=kr2[0:64, 1:2], in_=S4h[64:128, 3:4])
                    nc.vector.tensor_copy(out=kr2[64:128, 1:2], in_=S4h[0:64, 3:4])
                    sc = wk.tile([128, 2], f32, name=f"sc_{h}", tag=f"sc_{h}")
                    t3 = wk.tile([128, 2], f32, name=f"t3_{h}", tag=f"t3_{h}")
                    nc.vector.tensor_tensor(out=sc, in0=CA, in1=S4h[:, 0:2], op=A.mult)
                    nc.vector.scalar_tensor_tensor(out=sc, in0=CB, scalar=S4h[:, 2:3], in1=sc,
                                                   op0=A.mult, op1=A.add)
                    nc.vector.tensor_tensor(out=t3, in0=CC, in1=kr2, op=A.mult)
                    nc.vector.tensor_tensor(out=sc, in0=sc, in1=t3, op=A.add)
                    nc.vector.tensor_tensor(out=sc, in0=sc, in1=CD, op=A.add)
                    Dcol = wk.tile([128, 1], f32, name=f"D_{h}", tag=f"D_{h}")
                    Din = wk.tile([128, 1], f32, name=f"Di_{h}", tag=f"Di_{h}")
                    nc.vector.tensor_reduce(out=Dcol, in_=sc, axis=AX.X, op=A.add)
                    nc.vector.reciprocal(out=Din, in_=Dcol)
                    w2 = wk.tile([128, 2], f32, name=f"w2_{h}", tag=f"w2_{h}")
                    nc.vector.tensor_scalar(out=w2, in0=sc, scalar1=Din, scalar2=None, op0=A.mult)
                    t4 = wk.tile([128, 1], f32, name=f"t4_{h}", tag=f"t4_{h}")
                    nc.vector.tensor_scalar(out=uz6[:, 3 * h:3 * h + 1], in0=w2[:, 0:1],
                                            scalar1=bc[:, 2:3], scalar2=None, op0=A.mult)
                    nc.vector.tensor_scalar(out=uz6[:, 3 * h + 1:3 * h + 2], in0=w2[:, 1:2],
                                            scalar1=bc[:, 8:9], scalar2=None, op0=A.mult)
                    nc.vector.tensor_scalar(out=t4, in0=w2[:, 0:1], scalar1=bc[:, 5:6],
                                            scalar2=None, op0=A.mult)
                    nc.vector.scalar_tensor_tensor(out=uz6[:, 3 * h + 2:3 * h + 3], in0=w2[:, 1:2],
                                                   scalar=bc[:, 11:12], in1=t4,
                                                   op0=A.mult, op1=A.add)

                # base = z0*wos0 + z1*wos1 + bo  (no dependency on CC-B)
                base = wk.tile([128, 1024], f32, name="base", tag="base")
                nc.vector.scalar_tensor_tensor(out=base, in0=t_wos[:, 0, :], scalar=uz6[:, 2:3],
                                               in1=t_bo, op0=A.mult, op1=A.add)
                nc.vector.scalar_tensor_tensor(out=base, in0=t_wos[:, 1, :], scalar=uz6[:, 5:6],
                                               in1=base, op0=A.mult, op1=A.add)

                # ---- CC-B loadback + combine ----
                ys = wk.tile([128, 2048], f32, name="ys", tag="ys")
                if merged:
                    nc.sync.dma_start(out=ys[:, 0:1024], in_=ccm_out[pp][:, 0:1024])
                    nc.sync.dma_start(out=ys[:, 1024:2048], in_=ccm_out[pp][:, 1024:2048])
                elif fcc == f32:
                    nc.sync.dma_start(out=ys[:, 0:1024], in_=cc2_out[pp][:, 0:1024])
                    nc.sync.dma_start(out=ys[:, 1024:2048], in_=cc2_out[pp][:, 1024:2048])
                else:
                    ys16 = wk.tile([128, 2048], f16, name="ys16", tag="ys16")
                    nc.sync.dma_start(out=ys16[:, 0:1024], in_=cc2_out[pp][:, 0:1024])
                    nc.sync.dma_start(out=ys16[:, 1024:2048], in_=cc2_out[pp][:, 1024:2048])
                    nc.scalar.copy(out=ys[:, 0:1024], in_=ys16[:, 0:1024])
                    nc.scalar.copy(out=ys[:, 1024:2048], in_=ys16[:, 1024:2048])

                ysw = wk.tile([128, 2048], f32, name="ysw", tag="ysw")
                nc.vector.tensor_copy(out=ysw[0:64, :], in_=ys[64:128, :])
                nc.vector.tensor_copy(out=ysw[64:128, :], in_=ys[0:64, :])

                X = wk.tile([128, 1024], f32, name="X", tag="X")
                nc.vector.scalar_tensor_tensor(out=X, in0=ys[:, 0:1024], scalar=uz6[:, 0:1],
                                               in1=base, op0=A.mult, op1=A.add)
                nc.vector.scalar_tensor_tensor(out=X, in0=ys[:, 1024:2048], scalar=uz6[:, 3:4],
                                               in1=X, op0=A.mult, op1=A.add)
                nc.vector.scalar_tensor_tensor(out=X, in0=ysw[:, 0:1024], scalar=uz6[:, 1:2],
                                               in1=X, op0=A.mult, op1=A.add)
                nc.vector.scalar_tensor_tensor(out=X, in0=ysw[:, 1024:2048], scalar=uz6[:, 4:5],
                                               in1=X, op0=A.mult, op1=A.add)

                # ---- BN2 ----
                r2h = wk.tile([128, 2, 2], f32, name="r2h", tag="r2h")
                scr2 = wk.tile([128, 1024], f32, name="scr2", tag="scr2")
                for hh in range(2):
                    cols = slice(hh * 512, (hh + 1) * 512)
                    nc.vector.tensor_reduce(out=r2h[:, hh, 0:1], in_=X[:, cols], axis=AX.X, op=A.add)
                    nc.scalar.activation(out=scr2[:, cols], in_=X[:, cols], func=AF.Square,
                                         accum_out=r2h[:, hh, 1:2])
                r2 = wk.tile([128, 2], f32, name="r2", tag="r2")
                nc.vector.tensor_tensor(out=r2, in0=r2h[:, 0, :], in1=r2h[:, 1, :], op=A.add)
                st2_ps = ps.tile([128, 4], f32, name="st2", tag="sm")
                nc.tensor.matmul(out=st2_ps[0:2, 0:2], lhsT=t_mn, rhs=r2, start=True, stop=True)
                cst2 = wk.tile([2, 12], f32, name="cst2", tag="cst2")
                inv_n2 = 1.0 / float(B * DIM)
                nc.vector.tensor_scalar(out=cst2[:, 0:2], in0=st2_ps[0:2, 0:2], scalar1=inv_n2,
                                        scalar2=None, op0=A.mult)           # [mean, E2]
                nc.vector.tensor_tensor(out=cst2[:, 2:3], in0=cst2[:, 0:1], in1=cst2[:, 0:1], op=A.mult)
                nc.vector.tensor_tensor(out=cst2[:, 3:4], in0=cst2[:, 1:2], in1=cst2[:, 2:3], op=A.subtract)
                nc.scalar.activation(out=cst2[:, 4:5], in_=cst2[:, 3:4], func=AF.Sqrt,
                                     bias=eps_t, scale=1.0)
                nc.vector.reciprocal(out=cst2[:, 5:6], in_=cst2[:, 4:5])
                nc.vector.tensor_tensor(out=cst2[:, 6:7], in0=t_gb[0:2, 6:7], in1=cst2[:, 5:6], op=A.mult)  # abn
                nc.vector.tensor_tensor(out=cst2[:, 8:9], in0=cst2[:, 6:7], in1=cst2[:, 0:1], op=A.mult)
                nc.vector.tensor_tensor(out=cst2[:, 7:8], in0=t_gb[0:2, 7:8], in1=cst2[:, 8:9], op=A.subtract)  # cbn
                bc2_ps = ps.tile([128, 4], f32, name="bc2", tag="vr")
                nc.tensor.matmul(out=bc2_ps[:, 0:2], lhsT=t_m2, rhs=cst2[:, 6:8], start=True, stop=True)
                bc2 = wk.tile([128, 2], f32, name="bc2_sb", tag="bc2_sb")
                nc.scalar.copy(out=bc2, in_=bc2_ps[:, 0:2])
                fin = wk.tile([128, 1024], f32, name="fin", tag="fin")
                for hh in range(2):
                    cols = slice(hh * 512, (hh + 1) * 512)
                    nc.vector.tensor_scalar(out=fin[:, cols], in0=X[:, cols], scalar1=bc2[:, 0:1],
                                            scalar2=bc2[:, 1:2], op0=A.mult, op1=A.add)
                    nc.sync.dma_start(out=d_out[:, cols], in_=fin[:, cols])

    nc.compile()
    return nc


# --------------------------------------------------------------------------
# v6: software-pipelined emission — phase1(k+1) is emitted before phase2(k)
# so no engine queue ever blocks the next rep's collective inputs behind the
# current rep's post-collective tail.
# --------------------------------------------------------------------------

def _build_v6(mm_dt=None, reps=1, packed=False, slim=False, act_pack=False):
    import concourse.bass as bass
    import concourse.mybir as mybir
    import concourse.tile as tile
    from concourse import bacc

    f32 = mybir.dt.float32
    f16 = mybir.dt.float16
    i16 = mybir.dt.int16
    fmm = mybir.dt.float32r if (mm_dt or MM_DT) == "f32r" else mybir.dt.float32
    fcc = f32 if CC_F32 else f16
    A = mybir.AluOpType
    AF = mybir.ActivationFunctionType
    AX = mybir.AxisListType

    no_cc = NO_CC
    nc = bacc.Bacc(None, target_bir_lowering=False, debug=False, num_devices=NC)

    d_xt = nc.dram_tensor("xt", [128, 8, 128], fmm, kind="ExternalInput")
    d_wqk = nc.dram_tensor("wqk", [128, 8, 512], fmm, kind="ExternalInput")
    d_wv = nc.dram_tensor("wv", [128, 8, 256], f16, kind="ExternalInput")
    d_wo = nc.dram_tensor("wo", [128, 2, 1024], f16, kind="ExternalInput")
    d_wos = nc.dram_tensor("wos", [2, 1024], f32, kind="ExternalInput")
    d_bo = nc.dram_tensor("bo", [1024], f32, kind="ExternalInput")
    d_hm = nc.dram_tensor("hm", [128, 2], f32, kind="ExternalInput")
    d_mn = nc.dram_tensor("mn", [128, 2], f32, kind="ExternalInput")
    d_m2 = nc.dram_tensor("m2", [2, 128], f32, kind="ExternalInput")
    d_gb = nc.dram_tensor("gb", [2, 8], f32, kind="ExternalInput")
    d_out = nc.dram_tensor("out", [128, 1024], f32, kind="ExternalOutput")

    NCC = 2
    if packed:
        d_psc = nc.dram_tensor("psc", [128, 16], f32, kind="ExternalInput")
        d_ipsc = nc.dram_tensor("ipsc", [128, 16], f32, kind="ExternalInput")
        d_hmy = nc.dram_tensor("hmy", [128, 2], f32, kind="ExternalInput")
        d_isy = nc.dram_tensor("isy", [128, 1], f32, kind="ExternalInput")
        ccp_in = [nc.dram_tensor(f"ccp_in{i}", [128, 2080], i16) for i in range(NCC)]
        ccp_out = [nc.dram_tensor(f"ccp_out{i}", [128, 2080], i16, addr_space="Shared")
                   for i in range(NCC)]
    else:
        cc1_in = [nc.dram_tensor(f"cc1_in{i}", [128, 16], f32) for i in range(NCC)]
        cc1_out = [nc.dram_tensor(f"cc1_out{i}", [128, 16], f32, addr_space="Shared")
                   for i in range(NCC)]
        cc2_in = [nc.dram_tensor(f"cc2_in{i}", [128, 2048], fcc) for i in range(NCC)]
        cc2_out = [nc.dram_tensor(f"cc2_out{i}", [128, 2048], fcc, addr_space="Shared")
                   for i in range(NCC)]

    def bcast(ap, p=128):
        return bass.AP(tensor=ap.tensor, offset=ap.offset, ap=[[0, p]] + list(ap.ap))

    groups = [list(range(NC))]

    with tile.TileContext(nc) as tc:
        with (
            tc.tile_pool(name="const", bufs=2) as cst_pool,
            tc.tile_pool(name="work", bufs=1) as wk,
            tc.tile_pool(name="psum", bufs=1, space="PSUM") as ps,
            tc.tile_pool(name="psvt", bufs=2, space="PSUM") as psvt,
        ):
            def phase1(rep):
                pp = rep % NCC
                t_xt = [cst_pool.tile([128, 128], fmm, name=f"xt{c}", tag=f"xt{c}") for c in range(8)]
                t_wqk = [cst_pool.tile([128, 512], fmm, name=f"wqk{c}", tag=f"wqk{c}") for c in range(8)]
                t_wv = [cst_pool.tile([128, 256], f16, name=f"wv{c}", tag=f"wv{c}") for c in range(8)]
                t_wo = [cst_pool.tile([128, 1024], f16, name=f"wo{k}", tag=f"wo{k}") for k in range(2)]
                t_wos = cst_pool.tile([128, 2, 1024], f32, name="wos", tag="wos")
                t_bo = cst_pool.tile([128, 1024], f32, name="bo", tag="bo")
                t_hm = cst_pool.tile([128, 2], f32, name="hm", tag="hm")
                t_mn = cst_pool.tile([128, 2], f32, name="mn", tag="mn")
                t_m2 = cst_pool.tile([2, 128], f32, name="m2", tag="m2")
                t_gb = cst_pool.tile([2, 8], f32, name="gb", tag="gb")

                for c in range(8):
                    nc.sync.dma_start(out=t_xt[c], in_=d_xt[:, c, :])
                    nc.sync.dma_start(out=t_wqk[c], in_=d_wqk[:, c, :])
                for c in range(8):
                    nc.sync.dma_start(out=t_wv[c], in_=d_wv[:, c, :])
                for k in range(2):
                    nc.sync.dma_start(out=t_wo[k], in_=d_wo[:, k, :])
                nc.sync.dma_start(out=t_hm, in_=d_hm[:])
                nc.sync.dma_start(out=t_mn, in_=d_mn[:])
                nc.sync.dma_start(out=t_m2, in_=d_m2[:])
                nc.sync.dma_start(out=t_gb, in_=d_gb[:])
                nc.sync.dma_start(out=t_wos, in_=bcast(d_wos[:]))
                nc.sync.dma_start(out=t_bo, in_=bcast(d_bo[:]))
                if packed:
                    t_psc = cst_pool.tile([128, 16], f32, name="psc", tag="psc")
                    t_ipsc = cst_pool.tile([128, 16], f32, name="ipsc", tag="ipsc")
                    t_hmy = cst_pool.tile([128, 2], f32, name="hmy", tag="hmy")
                    t_isy = cst_pool.tile([128, 1], f32, name="isy", tag="isy")
                    nc.sync.dma_start(out=t_psc, in_=d_psc[:])
                    nc.sync.dma_start(out=t_ipsc, in_=d_ipsc[:])
                    nc.sync.dma_start(out=t_hmy, in_=d_hmy[:])
                    nc.sync.dma_start(out=t_isy, in_=d_isy[:])

                t_x16 = [wk.tile([128, 128], f16, name=f"x16_{c}", tag=f"x16_{c}", bufs=2)
                         for c in range(8)]
                for c in range(8):
                    nc.scalar.copy(out=t_x16[c], in_=t_xt[c])

                qk_ps = ps.tile([128, 512], f32, name="qk", tag="qk", bufs=2)
                for c in range(8):
                    nc.tensor.matmul(qk_ps, lhsT=t_xt[c], rhs=t_wqk[c],
                                     start=(c == 0), stop=(c == 7))
                vt_ps = [psvt.tile([128, 128], f32, name="vt", tag="vt") for _ in range(2)]
                for half in range(2):
                    for c in range(8):
                        nc.tensor.matmul(vt_ps[half], lhsT=t_wv[c][:, half * 128:(half + 1) * 128],
                                         rhs=t_x16[c], start=(c == 0), stop=(c == 7))

                # payload
                q_ap = qk_ps[:, 0:256]
                k_ap = qk_ps[:, 256:512]
                ksb = wk.tile([128, 256], f32, name="ksb", tag="ksb", bufs=2)
                nc.scalar.copy(out=ksb, in_=k_ap)
                ksw = wk.tile([128, 256], f32, name="ksw", tag="ksw", bufs=2)
                nc.vector.tensor_copy(out=ksw[0:64, :], in_=ksb[64:128, :])
                nc.vector.tensor_copy(out=ksw[64:128, :], in_=ksb[0:64, :])

                tmp4 = wk.tile([128, 4], f32, name="tmp4", tag="tmp4", bufs=2)
                prod1 = wk.tile([128, 256], f32, name="prod1", tag="prod1", bufs=2)
                prod2 = wk.tile([128, 256], f32, name="prod2", tag="prod2", bufs=2)
                nc.vector.tensor_tensor(out=prod1, in0=q_ap, in1=ksb, op=A.mult)
                nc.vector.tensor_reduce(out=tmp4[:, 0:1], in_=prod1, axis=AX.X, op=A.add)
                nc.vector.tensor_tensor(out=prod2, in0=q_ap, in1=ksw, op=A.mult)
                nc.vector.tensor_reduce(out=tmp4[:, 1:2], in_=prod2, axis=AX.X, op=A.add)
                nc.vector.tensor_reduce(out=tmp4[:, 2:4],
                                        in_=qk_ps[:].rearrange("p (t j) -> p t j", t=2),
                                        axis=AX.X, op=A.add)

                pay = wk.tile([128, 16], f32, name="pay", tag="pay", bufs=2)
                nc.vector.memset(pay, 0.0)
                nc.vector.tensor_scalar(out=pay[:, 0:4], in0=tmp4, scalar1=t_hm[:, 0:1],
                                        scalar2=None, op0=A.mult)
                nc.vector.tensor_scalar(out=pay[:, 4:8], in0=tmp4, scalar1=t_hm[:, 1:2],
                                        scalar2=None, op0=A.mult)
                sq1 = wk.tile([128, 256], f32, name="sq1", tag="sq1", bufs=2)
                sq2 = wk.tile([128, 256], f32, name="sq2", tag="sq2", bufs=2)
                nc.scalar.activation(out=sq1, in_=q_ap, func=AF.Square, accum_out=pay[:, 8:9])
                nc.scalar.activation(out=sq2, in_=k_ap, func=AF.Square, accum_out=pay[:, 9:10])
                vsq = [wk.tile([128, 128], f32, name=f"vsq{i}", tag=f"vsq{i}", bufs=2) for i in range(2)]
                v2ab = [wk.tile([128, 2], f32, name=f"v2ab{i}", tag=f"v2ab{i}", bufs=2) for i in range(2)]
                for half in range(2):
                    nc.scalar.activation(out=vsq[half], in_=vt_ps[half], func=AF.Square)
                    nc.vector.tensor_reduce(out=v2ab[half],
                                            in_=vsq[half][:].rearrange("p (t r) -> p t r", t=2),
                                            axis=AX.X, op=A.add)
                nc.vector.tensor_tensor(out=pay[:, 11:13], in0=v2ab[0], in1=v2ab[1], op=A.add)
                vsab = [wk.tile([128, 2], f32, name=f"vsab{i}", tag=f"vsab{i}", bufs=2) for i in range(2)]
                for half in range(2):
                    nc.vector.tensor_reduce(out=vsab[half],
                                            in_=vt_ps[half][:].rearrange("p (t r) -> p t r", t=2),
                                            axis=AX.X, op=A.add)
                nc.vector.tensor_tensor(out=pay[:, 13:15], in0=vsab[0], in1=vsab[1], op=A.add)

                if packed:
                    # fixed-point encode: v*s = hi*2048 + lo, exact int adds on wire
                    ps1 = wk.tile([128, 16], f32, name="ps1", tag="ps1", bufs=2)
                    nc.vector.tensor_tensor(out=ps1, in0=pay, in1=t_psc, op=A.mult)
                    payi = wk.tile([128, 32], i16, name="payi", tag="payi", bufs=2)
                    nc.vector.tensor_scalar(out=payi[:, 0:16], in0=ps1, scalar1=1.0 / 2048.0,
                                            scalar2=None, op0=A.mult)
                    hi_f = wk.tile([128, 16], f32, name="hi_f", tag="hi_f", bufs=2)
                    nc.vector.tensor_copy(out=hi_f, in_=payi[:, 0:16])
                    nc.vector.scalar_tensor_tensor(out=payi[:, 16:32], in0=hi_f, scalar=-2048.0,
                                                   in1=ps1, op0=A.mult, op1=A.add)
                    nc.sync.dma_start(out=ccp_in[pp][:, 2048:2080], in_=payi)
                else:
                    # CC-A issued as soon as the payload is staged
                    nc.sync.dma_start(out=cc1_in[pp][:], in_=pay)
                    if no_cc in (True, "no_a"):
                        nc.gpsimd.dma_start(out=cc1_out[pp][:], in_=cc1_in[pp][:])
                    else:
                        nc.gpsimd.collective_compute(
                            "AllReduce", A.add, replica_groups=groups,
                            ins=[cc1_in[pp][:]], outs=[cc1_out[pp][:]])

                # Y matmuls + head-pack + CC-B
                vts = wk.tile([128, 2, 128], f16, name="vts", tag="vts", bufs=2)
                for half in range(2):
                    nc.scalar.copy(out=vts[:, half, :], in_=vt_ps[half])
                y_ps = ps.tile([128, 1024], f32, name="y", tag="y")
                for k in range(2):
                    for nn in range(2):
                        nc.tensor.matmul(y_ps[:, nn * 512:(nn + 1) * 512],
                                         lhsT=vts[:, k, :], rhs=t_wo[k][:, nn * 512:(nn + 1) * 512],
                                         start=(k == 0), stop=(k == 1))
                if packed:
                    yh = wk.tile([128, 2048], i16, name="yh", tag="yh", bufs=2)
                    if act_pack:
                        nc.scalar.activation(out=yh[:, 0:1024], in_=y_ps, func=AF.Identity,
                                             scale=t_hmy[:, 0:1])
                        nc.scalar.activation(out=yh[:, 1024:2048], in_=y_ps, func=AF.Identity,
                                             scale=t_hmy[:, 1:2])
                    else:
                        nc.vector.tensor_scalar(out=yh[:, 0:1024], in0=y_ps, scalar1=t_hmy[:, 0:1],
                                                scalar2=None, op0=A.mult)
                        nc.vector.tensor_scalar(out=yh[:, 1024:2048], in0=y_ps, scalar1=t_hmy[:, 1:2],
                                                scalar2=None, op0=A.mult)
                    nc.sync.dma_start(out=ccp_in[pp][:, 0:2048], in_=yh)
                    if no_cc:
                        nc.gpsimd.dma_start(out=ccp_out[pp][:], in_=ccp_in[pp][:])
                    else:
                        nc.gpsimd.collective_compute(
                            "AllReduce", A.add, replica_groups=groups,
                            ins=[ccp_in[pp][:]], outs=[ccp_out[pp][:]],
                            cc_dim=CC_DIM, unique_tensors=CC_UT)
                else:
                    yh = wk.tile([128, 2048], fcc, name="yh", tag="yh", bufs=2)
                    nc.vector.tensor_scalar(out=yh[:, 0:1024], in0=y_ps, scalar1=t_hm[:, 0:1],
                                            scalar2=None, op0=A.mult)
                    nc.vector.tensor_scalar(out=yh[:, 1024:2048], in0=y_ps, scalar1=t_hm[:, 1:2],
                                            scalar2=None, op0=A.mult)
                    nc.sync.dma_start(out=cc2_in[pp][:], in_=yh)
                    if no_cc in (True, "no_b"):
                        nc.gpsimd.dma_start(out=cc2_out[pp][:], in_=cc2_in[pp][:])
                    else:
                        nc.gpsimd.collective_compute(
                            "AllReduce", A.add, replica_groups=groups,
                            ins=[cc2_in[pp][:]], outs=[cc2_out[pp][:]])

                ctx = dict(pp=pp, t_wos=t_wos, t_bo=t_bo, t_mn=t_mn, t_m2=t_m2,
                           t_gb=t_gb)
                if packed:
                    ctx.update(t_ipsc=t_ipsc, t_isy=t_isy)
                return ctx

            def phase2(ctx):
                pp = ctx["pp"]
                t_wos, t_bo = ctx["t_wos"], ctx["t_bo"]
                t_mn, t_m2, t_gb = ctx["t_mn"], ctx["t_m2"], ctx["t_gb"]

                # loadbacks (emitted after phase1 of the NEXT rep)
                S = wk.tile([128, 16], f32, name="S", tag="S", bufs=2)
                ys = wk.tile([128, 2048], f32, name="ys", tag="ys")
                if packed:
                    t_ipsc, t_isy = ctx["t_ipsc"], ctx["t_isy"]
                    Si = wk.tile([128, 32], i16, name="Si", tag="Si", bufs=2)
                    nc.sync.dma_start(out=Si, in_=ccp_out[pp][:, 2048:2080])
                    Shi = wk.tile([128, 16], f32, name="Shi", tag="Shi")
                    Slo = wk.tile([128, 16], f32, name="Slo", tag="Slo")
                    nc.vector.tensor_copy(out=Shi, in_=Si[:, 0:16])
                    nc.vector.tensor_copy(out=Slo, in_=Si[:, 16:32])
                    nc.vector.scalar_tensor_tensor(out=Slo, in0=Shi, scalar=2048.0, in1=Slo,
                                                   op0=A.mult, op1=A.add)
                    nc.vector.tensor_tensor(out=S, in0=Slo, in1=t_ipsc, op=A.mult)
                    ysi = wk.tile([128, 2048], i16, name="ysi", tag="ysi", bufs=2)
                    nc.sync.dma_start(out=ysi[:, 0:1024], in_=ccp_out[pp][:, 0:1024])
                    nc.sync.dma_start(out=ysi[:, 1024:2048], in_=ccp_out[pp][:, 1024:2048])
                    if not slim:
                        nc.vector.tensor_scalar(out=ys[:, 0:1024], in0=ysi[:, 0:1024],
                                                scalar1=t_isy[:, 0:1], scalar2=None, op0=A.mult)
                        nc.vector.tensor_scalar(out=ys[:, 1024:2048], in0=ysi[:, 1024:2048],
                                                scalar1=t_isy[:, 0:1], scalar2=None, op0=A.mult)
                elif fcc == f16:
                    nc.sync.dma_start(out=S, in_=cc1_out[pp][:])
                    ys16 = wk.tile([128, 2048], f16, name="ys16", tag="ys16", bufs=2)
                    nc.sync.dma_start(out=ys16[:, 0:1024], in_=cc2_out[pp][:, 0:1024])
                    nc.sync.dma_start(out=ys16[:, 1024:2048], in_=cc2_out[pp][:, 1024:2048])
                    nc.scalar.copy(out=ys[:, 0:1024], in_=ys16[:, 0:1024])
                    nc.scalar.copy(out=ys[:, 1024:2048], in_=ys16[:, 1024:2048])
                else:
                    nc.sync.dma_start(out=S, in_=cc1_out[pp][:])
                    nc.sync.dma_start(out=ys[:, 0:1024], in_=cc2_out[pp][:, 0:1024])
                    nc.sync.dma_start(out=ys[:, 1024:2048], in_=cc2_out[pp][:, 1024:2048])

                # BN1 stats
                rhs4 = wk.tile([128, 4], f32, name="rhs4", tag="rhs4")
                nc.vector.tensor_tensor(out=rhs4[:, 0:2], in0=S[:, 2:4], in1=S[:, 6:8], op=A.add)
                nc.vector.tensor_copy(out=rhs4[:, 2:4], in_=S[:, 8:10])
                ones1 = wk.tile([128, 1], f32, name="ones1", tag="ones1")
                nc.vector.memset(ones1, 1.0)
                st_ps = ps.tile([128, 16], f32, name="st", tag="sm")
                nc.tensor.matmul(out=st_ps[0:2, 0:4], lhsT=t_mn, rhs=rhs4, start=True, stop=True)
                nc.tensor.matmul(out=st_ps[0:2, 4:5], lhsT=S[:, 11:13], rhs=ones1,
                                 start=True, stop=True)
                nc.tensor.matmul(out=st_ps[0:2, 5:6], lhsT=S[:, 13:15], rhs=ones1,
                                 start=True, stop=True)
                sts = wk.tile([2, 6], f32, name="sts", tag="sts")
                nc.vector.tensor_copy(out=sts[:, 0:2], in_=st_ps[0:2, 0:2])
                nc.vector.tensor_copy(out=sts[:, 2:3], in_=st_ps[0:2, 5:6])
                nc.vector.tensor_copy(out=sts[:, 3:5], in_=st_ps[0:2, 2:4])
                nc.vector.tensor_copy(out=sts[:, 5:6], in_=st_ps[0:2, 4:5])

                cst = wk.tile([2, 32], f32, name="cst", tag="cst")
                eps_t = wk.tile([2, 1], f32, name="eps_t", tag="eps_t")
                nc.vector.memset(eps_t, EPS)
                inv_n1 = 1.0 / float(B * INNER)
                nc.vector.tensor_scalar(out=cst[:, 0:3], in0=sts[:, 0:3], scalar1=inv_n1,
                                        scalar2=None, op0=A.mult)
                nc.vector.tensor_scalar(out=cst[:, 3:6], in0=sts[:, 3:6], scalar1=inv_n1,
                                        scalar2=None, op0=A.mult)
                nc.vector.tensor_tensor(out=cst[:, 6:9], in0=cst[:, 0:3], in1=cst[:, 0:3], op=A.mult)
                nc.vector.tensor_tensor(out=cst[:, 9:12], in0=cst[:, 3:6], in1=cst[:, 6:9], op=A.subtract)
                nc.scalar.activation(out=cst[:, 12:15], in_=cst[:, 9:12], func=AF.Sqrt,
                                     bias=eps_t, scale=1.0)
                nc.vector.reciprocal(out=cst[:, 15:18], in_=cst[:, 12:15])
                nc.vector.tensor_tensor(out=cst[:, 18:21], in0=t_gb[0:2, 0:3], in1=cst[:, 15:18],
                                        op=A.mult)
                nc.vector.tensor_tensor(out=cst[:, 24:27], in0=cst[:, 18:21], in1=cst[:, 0:3],
                                        op=A.mult)
                nc.vector.tensor_tensor(out=cst[:, 21:24], in0=t_gb[0:2, 3:6], in1=cst[:, 24:27],
                                        op=A.subtract)

                bc_ps = ps.tile([128, 16], f32, name="bc", tag="sm")
                nc.tensor.matmul(out=bc_ps[:, 0:6], lhsT=t_m2, rhs=cst[:, 18:24],
                                 start=True, stop=True)
                bc = wk.tile([128, 12], f32, name="bc_sb", tag="bc_sb")
                nc.scalar.copy(out=bc[:, 0:6], in_=bc_ps[:, 0:6])
                nc.vector.tensor_copy(out=bc[0:64, 6:12], in_=bc[64:128, 0:6])
                nc.vector.tensor_copy(out=bc[64:128, 6:12], in_=bc[0:64, 0:6])

                CACD = wk.tile([128, 8], f32, name="CACD", tag="CACD")
                in0 = bass.AP(tensor=bc.tensor, offset=bc.offset,
                              ap=[list(bc.ap[0]), [3, 2], [0, 4]])
                in1 = bass.AP(tensor=bc.tensor, offset=bc.offset + 1,
                              ap=[list(bc.ap[0]), [0, 2], [3, 4]])
                nc.vector.tensor_tensor(out=CACD[:].rearrange("p (a b) -> p a b", a=2),
                                        in0=in0, in1=in1, op=A.mult)
                nc.vector.tensor_scalar(out=CACD[:, 5:6], in0=CACD[:, 5:6], scalar1=float(DIM),
                                        scalar2=None, op0=A.mult)
                nc.vector.tensor_scalar(out=CACD[:, 7:8], in0=CACD[:, 7:8], scalar1=float(DIM),
                                        scalar2=None, op0=A.mult)

                def cacd(k):
                    return bass.AP(tensor=CACD.tensor, offset=CACD.offset + k,
                                   ap=[list(CACD.ap[0]), [2, 2]])
                CA, CB, CC, CD = cacd(0), cacd(1), cacd(4), cacd(5)

                uz6 = wk.tile([128, 6], f32, name="uz6", tag="uz6")
                for h in range(2):
                    S4h = S[:, 4 * h:4 * h + 4]
                    kr2 = wk.tile([128, 2], f32, name=f"kr2_{h}", tag=f"kr2_{h}")
                    nc.vector.tensor_copy(out=kr2[:, 0:1], in_=S4h[:, 3:4])
                    nc.vector.tensor_copy(out=kr2[0:64, 1:2], in_=S4h[64:128, 3:4])
                    nc.vector.tensor_copy(out=kr2[64:128, 1:2], in_=S4h[0:64, 3:4])
                    sc = wk.tile([128, 2], f32, name=f"sc_{h}", tag=f"sc_{h}")
                    t3 = wk.tile([128, 2], f32, name=f"t3_{h}", tag=f"t3_{h}")
                    nc.vector.tensor_tensor(out=sc, in0=CA, in1=S4h[:, 0:2], op=A.mult)
                    nc.vector.scalar_tensor_tensor(out=sc, in0=CB, scalar=S4h[:, 2:3], in1=sc,
                                                   op0=A.mult, op1=A.add)
                    nc.vector.tensor_tensor(out=t3, in0=CC, in1=kr2, op=A.mult)
                    nc.vector.tensor_tensor(out=sc, in0=sc, in1=t3, op=A.add)
                    nc.vector.tensor_tensor(out=sc, in0=sc, in1=CD, op=A.add)
                    Dcol = wk.tile([128, 1], f32, name=f"D_{h}", tag=f"D_{h}")
                    Din = wk.tile([128, 1], f32, name=f"Di_{h}", tag=f"Di_{h}")
                    nc.vector.tensor_reduce(out=Dcol, in_=sc, axis=AX.X, op=A.add)
                    nc.vector.reciprocal(out=Din, in_=Dcol)
                    w2 = wk.tile([128, 2], f32, name=f"w2_{h}", tag=f"w2_{h}")
                    nc.vector.tensor_scalar(out=w2, in0=sc, scalar1=Din, scalar2=None, op0=A.mult)
                    t4 = wk.tile([128, 1], f32, name=f"t4_{h}", tag=f"t4_{h}")
                    nc.vector.tensor_scalar(out=uz6[:, 3 * h:3 * h + 1], in0=w2[:, 0:1],
                                            scalar1=bc[:, 2:3], scalar2=None, op0=A.mult)
                    nc.vector.tensor_scalar(out=uz6[:, 3 * h + 1:3 * h + 2], in0=w2[:, 1:2],
                                            scalar1=bc[:, 8:9], scalar2=None, op0=A.mult)
                    nc.vector.tensor_scalar(out=t4, in0=w2[:, 0:1], scalar1=bc[:, 5:6],
                                            scalar2=None, op0=A.mult)
                    nc.vector.scalar_tensor_tensor(out=uz6[:, 3 * h + 2:3 * h + 3], in0=w2[:, 1:2],
                                                   scalar=bc[:, 11:12], in1=t4,
                                                   op0=A.mult, op1=A.add)

                base = wk.tile([128, 1024], f32, name="base", tag="base")
                nc.vector.scalar_tensor_tensor(out=base, in0=t_wos[:, 0, :], scalar=uz6[:, 2:3],
                                               in1=t_bo, op0=A.mult, op1=A.add)
                nc.vector.scalar_tensor_tensor(out=base, in0=t_wos[:, 1, :], scalar=uz6[:, 5:6],
                                               in1=base, op0=A.mult, op1=A.add)

                if packed and slim:
                    # fold 1/sY into the per-row Y scalars; combine straight
                    # from int16 with converting reads
                    nc.vector.tensor_scalar(out=uz6[:, 0:2], in0=uz6[:, 0:2],
                                            scalar1=t_isy[:, 0:1], scalar2=None, op0=A.mult)
                    nc.vector.tensor_scalar(out=uz6[:, 3:5], in0=uz6[:, 3:5],
                                            scalar1=t_isy[:, 0:1], scalar2=None, op0=A.mult)
                    ysrc = ysi
                    ysw = wk.tile([128, 2048], i16, name="ysw", tag="ysw")
                else:
                    ysrc = ys
                    ysw = wk.tile([128, 2048], f32, name="ysw", tag="ysw")
                nc.vector.tensor_copy(out=ysw[0:64, :], in_=ysrc[64:128, :])
                nc.vector.tensor_copy(out=ysw[64:128, :], in_=ysrc[0:64, :])

                X = wk.tile([128, 1024], f32, name="X", tag="X")
                nc.vector.scalar_tensor_tensor(out=X, in0=ysrc[:, 0:1024], scalar=uz6[:, 0:1],
                                               in1=base, op0=A.mult, op1=A.add)
                nc.vector.scalar_tensor_tensor(out=X, in0=ysrc[:, 1024:2048], scalar=uz6[:, 3:4],
                                               in1=X, op0=A.mult, op1=A.add)
                nc.vector.scalar_tensor_tensor(out=X, in0=ysw[:, 0:1024], scalar=uz6[:, 1:2],
                                               in1=X, op0=A.mult, op1=A.add)
                nc.vector.scalar_tensor_tensor(out=X, in0=ysw[:, 1024:2048], scalar=uz6[:, 4:5],
                                               in1=X, op0=A.mult, op1=A.add)

                # BN2
                r2h = wk.tile([128, 2, 2], f32, name="r2h", tag="r2h")
                scr2 = wk.tile([128, 1024], f32, name="scr2", tag="scr2")
                for hh in range(2):
                    cols = slice(hh * 512, (hh + 1) * 512)
                    nc.vector.tensor_reduce(out=r2h[:, hh, 0:1], in_=X[:, cols], axis=AX.X, op=A.add)
                    nc.scalar.activation(out=scr2[:, cols], in_=X[:, cols], func=AF.Square,
                                         accum_out=r2h[:, hh, 1:2])
                r2 = wk.tile([128, 2], f32, name="r2", tag="r2")
                nc.vector.tensor_tensor(out=r2, in0=r2h[:, 0, :], in1=r2h[:, 1, :], op=A.add)
                st2_ps = ps.tile([128, 4], f32, name="st2", tag="sm")
                nc.tensor.matmul(out=st2_ps[0:2, 0:2], lhsT=t_mn, rhs=r2, start=True, stop=True)
                cst2 = wk.tile([2, 12], f32, name="cst2", tag="cst2")
                inv_n2 = 1.0 / float(B * DIM)
                nc.vector.tensor_scalar(out=cst2[:, 0:2], in0=st2_ps[0:2, 0:2], scalar1=inv_n2,
                                        scalar2=None, op0=A.mult)
                nc.vector.tensor_tensor(out=cst2[:, 2:3], in0=cst2[:, 0:1], in1=cst2[:, 0:1], op=A.mult)
                nc.vector.tensor_tensor(out=cst2[:, 3:4], in0=cst2[:, 1:2], in1=cst2[:, 2:3], op=A.subtract)
                nc.scalar.activation(out=cst2[:, 4:5], in_=cst2[:, 3:4], func=AF.Sqrt,
                                     bias=eps_t, scale=1.0)
                nc.vector.reciprocal(out=cst2[:, 5:6], in_=cst2[:, 4:5])
                nc.vector.tensor_tensor(out=cst2[:, 6:7], in0=t_gb[0:2, 6:7], in1=cst2[:, 5:6], op=A.mult)
                nc.vector.tensor_tensor(out=cst2[:, 8:9], in0=cst2[:, 6:7], in1=cst2[:, 0:1], op=A.mult)
                nc.vector.tensor_tensor(out=cst2[:, 7:8], in0=t_gb[0:2, 7:8], in1=cst2[:, 8:9], op=A.subtract)
                bc2_ps = ps.tile([128, 4], f32, name="bc2", tag="sm")
                nc.tensor.matmul(out=bc2_ps[:, 0:2], lhsT=t_m2, rhs=cst2[:, 6:8], start=True, stop=True)
                bc2 = wk.tile([128, 2], f32, name="bc2_sb", tag="bc2_sb")
                nc.scalar.copy(out=bc2, in_=bc2_ps[:, 0:2])
                fin = wk.tile([128, 1024], f32, name="fin", tag="fin")
                for hh in range(2):
                    cols = slice(hh * 512, (hh + 1) * 512)
                    if slim:
                        nc.scalar.activation(out=fin[:, cols], in_=X[:, cols], func=AF.Identity,
                                             scale=bc2[:, 0:1], bias=bc2[:, 1:2])
                    else:
                        nc.vector.tensor_scalar(out=fin[:, cols], in0=X[:, cols], scalar1=bc2[:, 0:1],
                                                scalar2=bc2[:, 1:2], op0=A.mult, op1=A.add)
                    nc.sync.dma_start(out=d_out[:, cols], in_=fin[:, cols])

            ctxs = []
            for rep in range(reps):
                ctxs.append(phase1(rep))
                if rep >= 1:
                    phase2(ctxs[rep - 1])
            phase2(ctxs[-1])

    nc.compile()
    return nc


def _prep_inputs_v2(x, Wq, Wk, Wv, Wo, bo, g_q, b_q, g_k, b_k, g_v, b_v, g_bn, b_bn):
    f = np.float32
    f16 = np.float16
    x, Wq, Wk, Wv, Wo, bo = (np.asarray(t, f) for t in (x, Wq, Wk, Wv, Wo, bo))
    g_q, b_q, g_k, b_k, g_v, b_v, g_bn, b_bn = (
        np.asarray(t, f) for t in (g_q, b_q, g_k, b_k, g_v, b_v, g_bn, b_bn))
    xf = np.ascontiguousarray(x, f).reshape(B, N, DIM)
    Xr = np.ascontiguousarray(xf.transpose(1, 0, 2).reshape(N * B, DIM))   # n-major rows
    xt = np.ascontiguousarray(Xr.T.reshape(8, 128, 128).transpose(1, 0, 2))  # [p, c, r]

    mn = np.zeros((128, 2), f)
    mn[0:64, 0] = 1.0
    mn[64:128, 1] = 1.0
    m2 = np.ascontiguousarray(mn.T)            # (2, 128)
    gb = np.stack([g_q, g_k, g_v, b_q, b_k, b_v, g_bn, b_bn], axis=1).astype(f)
    wos = np.stack([Wo[:, 0:DIM].sum(1), Wo[:, DIM:INNER].sum(1)]).astype(f)  # (2, 1024)

    in_maps = []
    for i in range(NC):
        rows = slice(i * DPC, (i + 1) * DPC)
        head = i // 4
        wqk_c = np.concatenate([Wq[rows], Wk[rows]], axis=0).astype(f)       # (512, 1024)
        wqk = np.ascontiguousarray(wqk_c.T.reshape(8, 128, 512).transpose(1, 0, 2))
        wv_c = np.asarray(Wv[rows], f16)                                      # (256, 1024)
        wv = np.ascontiguousarray(wv_c.T.reshape(8, 128, 256).transpose(1, 0, 2))
        WoC = np.asarray(Wo[:, rows], f16)                                    # (1024, 256)
        wo = np.ascontiguousarray(WoC.T.reshape(2, 128, 1024).transpose(1, 0, 2))
        hm = np.zeros((128, 2), f)
        hm[:, head] = 1.0
        in_maps.append({
            "xt": xt, "wqk": wqk, "wv": wv, "wo": wo,
            "wos": wos, "bo": bo, "hm": hm, "mn": mn, "m2": m2, "gb": gb,
        })
    return in_maps


# --------------------------------------------------------------------------
# v1 (legacy, known-good): serial AllGather -> combine -> AllReduce, all f32
# --------------------------------------------------------------------------

def _build_v1(mm_dt=None, reps=1):
    import concourse.bass as bass
    import concourse.mybir as mybir
    import concourse.tile as tile
    from concourse import bacc

    f32 = mybir.dt.float32
    fmm = mybir.dt.float32r if (mm_dt or MM_DT) == "f32r" else mybir.dt.float32
    A = mybir.AluOpType
    AF = mybir.ActivationFunctionType
    AX = mybir.AxisListType

    no_cc = NO_CC
    nc = bacc.Bacc(None, target_bir_lowering=False, debug=False, num_devices=NC)

    # ---- I/O ----
    d_xt = nc.dram_tensor("xt", [128, 8, 128], fmm, kind="ExternalInput")
    d_wqk = nc.dram_tensor("wqk", [128, 8, 512], fmm, kind="ExternalInput")
    d_wv = nc.dram_tensor("wv", [128, 8, 256], fmm, kind="ExternalInput")
    d_wo = nc.dram_tensor("wo", [128, 2, 1024], fmm, kind="ExternalInput")
    d_wos = nc.dram_tensor("wos", [1024], f32, kind="ExternalInput")
    d_bo8 = nc.dram_tensor("bo8", [1024], f32, kind="ExternalInput")
    d_hm = nc.dram_tensor("hm", [128, 2], f32, kind="ExternalInput")
    d_mn = nc.dram_tensor("mn", [128, 2], f32, kind="ExternalInput")
    d_m2 = nc.dram_tensor("m2", [2, 128], f32, kind="ExternalInput")
    d_m2o = nc.dram_tensor("m2o", [2, 128], f32, kind="ExternalInput")
    d_gb = nc.dram_tensor("gb", [2, 8], f32, kind="ExternalInput")
    d_out = nc.dram_tensor("out", [128, 1024], f32, kind="ExternalOutput")

    cc1_in = nc.dram_tensor("cc1_in", [128, 16], f32)
    cc1_out = nc.dram_tensor("cc1_out", [NC * 128, 16], f32, addr_space="Shared")
    cc2_in = nc.dram_tensor("cc2_in", [128, 1024], f32)
    cc2_out = nc.dram_tensor("cc2_out", [128, 1024], f32, addr_space="Shared")

    def bcast(ap, p=128):
        return bass.AP(tensor=ap.tensor, offset=ap.offset, ap=[[0, p]] + list(ap.ap))

    groups = [list(range(NC))]

    with tile.TileContext(nc) as tc:
        with (
            tc.tile_pool(name="const", bufs=1) as cst_pool,
            tc.tile_pool(name="work", bufs=1) as wk,
            tc.tile_pool(name="psum", bufs=1, space="PSUM") as ps,
            tc.tile_pool(name="psvt", bufs=2, space="PSUM") as psvt,
        ):
            for _rep in range(reps):
                # ---- constant loads (chunked for DMA/compute overlap) ----
                t_xt = [cst_pool.tile([128, 128], fmm, name=f"xt{c}", tag=f"xt{c}") for c in range(8)]
                t_wqk = [cst_pool.tile([128, 512], fmm, name=f"wqk{c}", tag=f"wqk{c}") for c in range(8)]
                t_wv = [cst_pool.tile([128, 256], fmm, name=f"wv{c}", tag=f"wv{c}") for c in range(8)]
                t_wo = [cst_pool.tile([128, 1024], fmm, name=f"wo{k}", tag=f"wo{k}") for k in range(2)]
                t_wos = cst_pool.tile([128, 1024], f32, name="wos", tag="wos")
                t_bo8 = cst_pool.tile([128, 1024], f32, name="bo8", tag="bo8")
                t_hm = cst_pool.tile([128, 2], f32, name="hm", tag="hm")
                t_mn = cst_pool.tile([128, 2], f32, name="mn", tag="mn")
                t_m2 = cst_pool.tile([2, 128], f32, name="m2", tag="m2")
                t_m2o = cst_pool.tile([2, 128], f32, name="m2o", tag="m2o")
                t_gb = cst_pool.tile([2, 8], f32, name="gb", tag="gb")

                for c in range(8):
                    nc.sync.dma_start(out=t_xt[c], in_=d_xt[:, c, :])
                    nc.sync.dma_start(out=t_wqk[c], in_=d_wqk[:, c, :])
                    nc.sync.dma_start(out=t_wv[c], in_=d_wv[:, c, :])
                for k in range(2):
                    nc.sync.dma_start(out=t_wo[k], in_=d_wo[:, k, :])
                nc.sync.dma_start(out=t_hm, in_=d_hm[:])
                nc.sync.dma_start(out=t_mn, in_=d_mn[:])
                nc.sync.dma_start(out=t_m2, in_=d_m2[:])
                nc.sync.dma_start(out=t_m2o, in_=d_m2o[:])
                nc.sync.dma_start(out=t_gb, in_=d_gb[:])
                nc.sync.dma_start(out=t_wos, in_=bcast(d_wos[:]))
                nc.sync.dma_start(out=t_bo8, in_=bcast(d_bo8[:]))

                # ---- phase 1: projections ----
                qk_ps = ps.tile([128, 512], f32, name="qk", tag="qk")
                for c in range(8):
                    nc.tensor.matmul(qk_ps, lhsT=t_xt[c], rhs=t_wqk[c],
                                     start=(c == 0), stop=(c == 7))
                vt_ps = [psvt.tile([128, 128], f32, name="vt", tag="vt") for _ in range(2)]
                for half in range(2):
                    for c in range(8):
                        nc.tensor.matmul(vt_ps[half], lhsT=t_wv[c][:, half * 128:(half + 1) * 128],
                                         rhs=t_xt[c], start=(c == 0), stop=(c == 7))
                vts = wk.tile([128, 2, 128], fmm, name="vts", tag="vts")
                for half in range(2):
                    nc.scalar.copy(out=vts[:, half, :], in_=vt_ps[half])

                # ---- phase 2: payload (per-chunk partial sums) ----
                q_ap = qk_ps[:, 0:256]
                k_ap = qk_ps[:, 256:512]
                ksb = wk.tile([128, 256], f32, name="ksb", tag="ksb")
                nc.scalar.copy(out=ksb, in_=k_ap)
                ksw = wk.tile([128, 256], f32, name="ksw", tag="ksw")
                nc.vector.tensor_copy(out=ksw[0:64, :], in_=ksb[64:128, :])
                nc.vector.tensor_copy(out=ksw[64:128, :], in_=ksb[0:64, :])

                tmp4 = wk.tile([128, 4], f32, name="tmp4", tag="tmp4")
                prod1 = wk.tile([128, 256], f32, name="prod1", tag="prod1")
                prod2 = wk.tile([128, 256], f32, name="prod2", tag="prod2")
                nc.vector.tensor_tensor(out=prod1, in0=q_ap, in1=ksb, op=A.mult)
                nc.vector.tensor_reduce(out=tmp4[:, 0:1], in_=prod1, axis=AX.X, op=A.add)
                nc.vector.tensor_tensor(out=prod2, in0=q_ap, in1=ksw, op=A.mult)
                nc.vector.tensor_reduce(out=tmp4[:, 1:2], in_=prod2, axis=AX.X, op=A.add)
                nc.vector.tensor_reduce(out=tmp4[:, 2:4],
                                        in_=qk_ps[:].rearrange("p (t j) -> p t j", t=2),
                                        axis=AX.X, op=A.add)

                pay = wk.tile([128, 16], f32, name="pay", tag="pay")
                nc.vector.memset(pay, 0.0)
                nc.vector.tensor_scalar(out=pay[:, 0:4], in0=tmp4, scalar1=t_hm[:, 0:1],
                                        scalar2=None, op0=A.mult)
                nc.vector.tensor_scalar(out=pay[:, 4:8], in0=tmp4, scalar1=t_hm[:, 1:2],
                                        scalar2=None, op0=A.mult)
                sq1 = wk.tile([128, 256], f32, name="sq1", tag="sq1")
                sq2 = wk.tile([128, 256], f32, name="sq2", tag="sq2")
                nc.scalar.activation(out=sq1, in_=q_ap, func=AF.Square, accum_out=pay[:, 8:9])
                nc.scalar.activation(out=sq2, in_=k_ap, func=AF.Square, accum_out=pay[:, 9:10])
                vsq = [wk.tile([128, 128], f32, name=f"vsq{i}", tag=f"vsq{i}") for i in range(2)]
                v2ab = [wk.tile([128, 2], f32, name=f"v2ab{i}", tag=f"v2ab{i}") for i in range(2)]
                for half in range(2):
                    nc.scalar.activation(out=vsq[half], in_=vt_ps[half], func=AF.Square)
                    nc.vector.tensor_reduce(out=v2ab[half],
                                            in_=vsq[half][:].rearrange("p (t r) -> p t r", t=2),
                                            axis=AX.X, op=A.add)
                nc.vector.tensor_tensor(out=pay[:, 11:13], in0=v2ab[0], in1=v2ab[1], op=A.add)
                vsab = [wk.tile([128, 2], f32, name=f"vsab{i}", tag=f"vsab{i}") for i in range(2)]
                for half in range(2):
                    nc.vector.tensor_reduce(out=vsab[half],
                                            in_=vt_ps[half][:].rearrange("p (t r) -> p t r", t=2),
                                            axis=AX.X, op=A.add)
                nc.vector.tensor_tensor(out=pay[:, 13:15], in0=vsab[0], in1=vsab[1], op=A.add)

                # ---- collective 1: AllGather payload, local sum ----
                nc.sync.dma_start(out=cc1_in[:], in_=pay)
                if no_cc is True:
                    nc.gpsimd.dma_start(out=cc1_out[0:128, :], in_=cc1_in[:])
                else:
                    nc.gpsimd.collective_compute(
                        "AllGather", A.bypass, replica_groups=groups,
                        ins=[cc1_in[:]], outs=[cc1_out[:]])
                gat = wk.tile([128, 8, 16], f32, name="gat", tag="gat")
                nc.sync.dma_start(out=gat, in_=cc1_out[:].rearrange("(c p) f -> p c f", p=128))
                S = wk.tile([128, 16], f32, name="S", tag="S")
                nc.vector.tensor_reduce(out=S, in_=gat[:].rearrange("p c f -> p f c"),
                                        axis=AX.X, op=A.add)

                # ---- Y matmuls (overlap the AllGather): Y = V @ WoC.T ----
                y_ps = ps.tile([128, 1024], f32, name="y", tag="y")
                for k in range(2):
                    for nn in range(2):
                        nc.tensor.matmul(y_ps[:, nn * 512:(nn + 1) * 512],
                                         lhsT=vts[:, k, :], rhs=t_wo[k][:, nn * 512:(nn + 1) * 512],
                                         start=(k == 0), stop=(k == 1))
                ysw_sb = wk.tile([128, 1024], f32, name="ysw_sb", tag="ysw_sb")
                nc.vector.tensor_copy(out=ysw_sb[0:64, :], in_=y_ps[64:128, :])
                nc.vector.tensor_copy(out=ysw_sb[64:128, :], in_=y_ps[0:64, :])

                # ---- post-gather: head-slot select ----
                S4 = wk.tile([128, 4], f32, name="S4", tag="S4")
                th = wk.tile([128, 4], f32, name="th", tag="th")
                nc.vector.tensor_scalar(out=th, in0=S[:, 0:4], scalar1=t_hm[:, 0:1],
                                        scalar2=None, op0=A.mult)
                nc.vector.scalar_tensor_tensor(out=S4, in0=S[:, 4:8], scalar=t_hm[:, 1:2],
                                               in1=th, op0=A.mult, op1=A.add)
                kr2 = wk.tile([128, 2], f32, name="kr2", tag="kr2")
                nc.vector.tensor_copy(out=kr2[:, 0:1], in_=S4[:, 3:4])
                nc.vector.tensor_copy(out=kr2[0:64, 1:2], in_=S4[64:128, 3:4])
                nc.vector.tensor_copy(out=kr2[64:128, 1:2], in_=S4[0:64, 3:4])

                # ---- global BN1 stats ----
                rhs4 = wk.tile([128, 4], f32, name="rhs4", tag="rhs4")
                nc.vector.tensor_tensor(out=rhs4[:, 0:2], in0=S[:, 2:4], in1=S[:, 6:8], op=A.add)
                nc.vector.tensor_copy(out=rhs4[:, 2:4], in_=S[:, 8:10])
                ones1 = wk.tile([128, 1], f32, name="ones1", tag="ones1")
                nc.vector.memset(ones1, 1.0)
                st_ps = ps.tile([128, 16], f32, name="st", tag="qk")
                nc.tensor.matmul(out=st_ps[0:2, 0:4], lhsT=t_mn, rhs=rhs4, start=True, stop=True)
                nc.tensor.matmul(out=st_ps[0:2, 4:5], lhsT=S[:, 11:13], rhs=ones1,
                                 start=True, stop=True)
                nc.tensor.matmul(out=st_ps[0:2, 5:6], lhsT=S[:, 13:15], rhs=ones1,
                                 start=True, stop=True)
                sts = wk.tile([2, 6], f32, name="sts", tag="sts")   # [Sq Sk Sv SSq SSk SSv]
                nc.vector.tensor_copy(out=sts[:, 0:2], in_=st_ps[0:2, 0:2])
                nc.vector.tensor_copy(out=sts[:, 2:3], in_=st_ps[0:2, 5:6])
                nc.vector.tensor_copy(out=sts[:, 3:5], in_=st_ps[0:2, 2:4])
                nc.vector.tensor_copy(out=sts[:, 5:6], in_=st_ps[0:2, 4:5])

                cst = wk.tile([2, 32], f32, name="cst", tag="cst")
                eps_t = wk.tile([2, 1], f32, name="eps_t", tag="eps_t")
                nc.vector.memset(eps_t, EPS)
                inv_n1 = 1.0 / float(B * INNER)
                nc.vector.tensor_scalar(out=cst[:, 0:3], in0=sts[:, 0:3], scalar1=inv_n1,
                                        scalar2=None, op0=A.mult)          # means
                nc.vector.tensor_scalar(out=cst[:, 3:6], in0=sts[:, 3:6], scalar1=inv_n1,
                                        scalar2=None, op0=A.mult)          # E[x^2]
                nc.vector.tensor_tensor(out=cst[:, 6:9], in0=cst[:, 0:3], in1=cst[:, 0:3], op=A.mult)
                nc.vector.tensor_tensor(out=cst[:, 9:12], in0=cst[:, 3:6], in1=cst[:, 6:9], op=A.subtract)
                nc.scalar.activation(out=cst[:, 12:15], in_=cst[:, 9:12], func=AF.Sqrt,
                                     bias=eps_t, scale=1.0)
                nc.vector.reciprocal(out=cst[:, 15:18], in_=cst[:, 12:15])
                nc.vector.tensor_tensor(out=cst[:, 18:21], in0=t_gb[0:2, 0:3], in1=cst[:, 15:18],
                                        op=A.mult)                          # A = g*rstd
                nc.vector.tensor_tensor(out=cst[:, 24:27], in0=cst[:, 18:21], in1=cst[:, 0:3],
                                        op=A.mult)                          # A*mean
                nc.vector.tensor_tensor(out=cst[:, 21:24], in0=t_gb[0:2, 3:6], in1=cst[:, 24:27],
                                        op=A.subtract)                      # C = b - A*mean

                bc_ps = ps.tile([128, 16], f32, name="bc", tag="vr")
                nc.tensor.matmul(out=bc_ps[:, 0:6], lhsT=t_m2, rhs=cst[:, 18:24],
                                 start=True, stop=True)
                nc.tensor.matmul(out=bc_ps[:, 6:12], lhsT=t_m2o, rhs=cst[:, 18:24],
                                 start=True, stop=True)
                bc = wk.tile([128, 12], f32, name="bc_sb", tag="bc_sb")
                nc.scalar.copy(out=bc, in_=bc_ps[:, 0:12])
                # bc cols: 0 Aq 1 Ak 2 Av 3 Cq 4 Ck 5 Cv | 6 Aq' 7 Ak' 8 Av' 9 Cq' 10 Ck' 11 Cv'

                # ---- scores ----
                CACD = wk.tile([128, 8], f32, name="CACD", tag="CACD")
                in0 = bass.AP(tensor=bc.tensor, offset=bc.offset,
                              ap=[list(bc.ap[0]), [3, 2], [0, 4]])       # [Aq x4, Cq x4]
                in1 = bass.AP(tensor=bc.tensor, offset=bc.offset + 1,
                              ap=[list(bc.ap[0]), [0, 2], [3, 4]])       # [Ak Ck Ak' Ck'] x2
                nc.vector.tensor_tensor(out=CACD[:].rearrange("p (a b) -> p a b", a=2),
                                        in0=in0, in1=in1, op=A.mult)
                nc.vector.tensor_scalar(out=CACD[:, 5:6], in0=CACD[:, 5:6], scalar1=float(DIM),
                                        scalar2=None, op0=A.mult)
                nc.vector.tensor_scalar(out=CACD[:, 7:8], in0=CACD[:, 7:8], scalar1=float(DIM),
                                        scalar2=None, op0=A.mult)

                def cacd(k):
                    return bass.AP(tensor=CACD.tensor, offset=CACD.offset + k,
                                   ap=[list(CACD.ap[0]), [2, 2]])
                CA, CB, CC, CD = cacd(0), cacd(1), cacd(4), cacd(5)

                sc = wk.tile([128, 2], f32, name="sc", tag="sc")
                t3 = wk.tile([128, 2], f32, name="t3", tag="t3")
                nc.vector.tensor_tensor(out=sc, in0=CA, in1=S4[:, 0:2], op=A.mult)
                nc.vector.scalar_tensor_tensor(out=sc, in0=CB, scalar=S4[:, 2:3], in1=sc,
                                               op0=A.mult, op1=A.add)
                nc.vector.tensor_tensor(out=t3, in0=CC, in1=kr2, op=A.mult)
                nc.vector.tensor_tensor(out=sc, in0=sc, in1=t3, op=A.add)
                nc.vector.tensor_tensor(out=sc, in0=sc, in1=CD, op=A.add)

                Dcol = wk.tile([128, 1], f32, name="Dcol", tag="Dcol")
                Din = wk.tile([128, 1], f32, name="Din", tag="Din")
                nc.vector.tensor_reduce(out=Dcol, in_=sc, axis=AX.X, op=A.add)
                nc.vector.reciprocal(out=Din, in_=Dcol)
                w2 = wk.tile([128, 2], f32, name="w2", tag="w2")
                nc.vector.tensor_scalar(out=w2, in0=sc, scalar1=Din, scalar2=None, op0=A.mult)
                uz = wk.tile([128, 3], f32, name="uz", tag="uz")   # [u_diag, u_off, z]
                t4 = wk.tile([128, 1], f32, name="t4", tag="t4")
                nc.vector.tensor_scalar(out=uz[:, 0:1], in0=w2[:, 0:1], scalar1=bc[:, 2:3], scalar2=None, op0=A.mult)
                nc.vector.tensor_scalar(out=uz[:, 1:2], in0=w2[:, 1:2], scalar1=bc[:, 8:9], scalar2=None, op0=A.mult)
                nc.vector.tensor_scalar(out=t4, in0=w2[:, 0:1], scalar1=bc[:, 5:6], scalar2=None, op0=A.mult)
                nc.vector.scalar_tensor_tensor(out=uz[:, 2:3], in0=w2[:, 1:2], scalar=bc[:, 11:12],
                                               in1=t4, op0=A.mult, op1=A.add)

                # ---- combine: partial = u_diag*Y + u_off*Ysw + z*wo_sum + bo/8 ----
                Rt = wk.tile([128, 1024], f32, name="Rt", tag="Rt")
                nc.vector.scalar_tensor_tensor(out=Rt, in0=t_wos, scalar=uz[:, 2:3], in1=t_bo8,
                                               op0=A.mult, op1=A.add)
                t2 = wk.tile([128, 1024], f32, name="t2", tag="t2")
                nc.vector.scalar_tensor_tensor(out=t2, in0=ysw_sb, scalar=uz[:, 1:2], in1=Rt,
                                               op0=A.mult, op1=A.add)
                outp = wk.tile([128, 1024], f32, name="outp", tag="outp")
                nc.vector.scalar_tensor_tensor(out=outp, in0=y_ps, scalar=uz[:, 0:1], in1=t2,
                                               op0=A.mult, op1=A.add)

                # ---- collective 2: AllReduce partials ----
                nc.sync.dma_start(out=cc2_in[:], in_=outp)
                if no_cc in (True, "no_ar"):
                    nc.gpsimd.dma_start(out=cc2_out[:], in_=cc2_in[:])
                else:
                    nc.gpsimd.collective_compute(
                        "AllReduce", A.add, replica_groups=groups,
                        ins=[cc2_in[:]], outs=[cc2_out[:]])
                Xt = wk.tile([128, 1024], f32, name="Xt", tag="Xt")
                nc.sync.dma_start(out=Xt[:, 0:512], in_=cc2_out[:, 0:512])
                nc.sync.dma_start(out=Xt[:, 512:1024], in_=cc2_out[:, 512:1024])

                # ---- BN2 (halves overlap the Xt DMA) ----
                r2h = wk.tile([128, 2, 2], f32, name="r2h", tag="r2h")
                scr2 = wk.tile([128, 1024], f32, name="scr2", tag="scr2")
                for hh in range(2):
                    cols = slice(hh * 512, (hh + 1) * 512)
                    nc.vector.tensor_reduce(out=r2h[:, hh, 0:1], in_=Xt[:, cols], axis=AX.X, op=A.add)
                    nc.scalar.activation(out=scr2[:, cols], in_=Xt[:, cols], func=AF.Square,
                                         accum_out=r2h[:, hh, 1:2])
                r2 = wk.tile([128, 2], f32, name="r2", tag="r2")
                nc.vector.tensor_tensor(out=r2, in0=r2h[:, 0, :], in1=r2h[:, 1, :], op=A.add)
                st2_ps = ps.tile([128, 4], f32, name="st2", tag="qk")
                nc.tensor.matmul(out=st2_ps[0:2, 0:2], lhsT=t_mn, rhs=r2, start=True, stop=True)
                cst2 = wk.tile([2, 12], f32, name="cst2", tag="cst2")
                inv_n2 = 1.0 / float(B * DIM)
                nc.vector.tensor_scalar(out=cst2[:, 0:2], in0=st2_ps[0:2, 0:2], scalar1=inv_n2,
                                        scalar2=None, op0=A.mult)           # [mean, E2]
                nc.vector.tensor_tensor(out=cst2[:, 2:3], in0=cst2[:, 0:1], in1=cst2[:, 0:1], op=A.mult)
                nc.vector.tensor_tensor(out=cst2[:, 3:4], in0=cst2[:, 1:2], in1=cst2[:, 2:3], op=A.subtract)
                nc.scalar.activation(out=cst2[:, 4:5], in_=cst2[:, 3:4], func=AF.Sqrt,
                                     bias=eps_t, scale=1.0)
                nc.vector.reciprocal(out=cst2[:, 5:6], in_=cst2[:, 4:5])
                nc.vector.tensor_tensor(out=cst2[:, 6:7], in0=t_gb[0:2, 6:7], in1=cst2[:, 5:6], op=A.mult)  # abn
                nc.vector.tensor_tensor(out=cst2[:, 8:9], in0=cst2[:, 6:7], in1=cst2[:, 0:1], op=A.mult)
                nc.vector.tensor_tensor(out=cst2[:, 7:8], in0=t_gb[0:2, 7:8], in1=cst2[:, 8:9], op=A.subtract)  # cbn
                bc2_ps = ps.tile([128, 4], f32, name="bc2", tag="vr")
                nc.tensor.matmul(out=bc2_ps[:, 0:2], lhsT=t_m2, rhs=cst2[:, 6:8], start=True, stop=True)
                bc2 = wk.tile([128, 2], f32, name="bc2_sb", tag="bc2_sb")
                nc.scalar.copy(out=bc2, in_=bc2_ps[:, 0:2])
                fin = wk.tile([128, 1024], f32, name="fin", tag="fin")
                for hh in range(2):
                    cols = slice(hh * 512, (hh + 1) * 512)
                    nc.vector.tensor_scalar(out=fin[:, cols], in0=Xt[:, cols], scalar1=bc2[:, 0:1],
                                            scalar2=bc2[:, 1:2], op0=A.mult, op1=A.add)
                    nc.sync.dma_start(out=d_out[:, cols], in_=fin[:, cols])

    nc.compile()
    return nc


def _prep_inputs_v1(x, Wq, Wk, Wv, Wo, bo, g_q, b_q, g_k, b_k, g_v, b_v, g_bn, b_bn):
    f = np.float32
    x, Wq, Wk, Wv, Wo, bo = (np.asarray(t, f) for t in (x, Wq, Wk, Wv, Wo, bo))
    g_q, b_q, g_k, b_k, g_v, b_v, g_bn, b_bn = (
        np.asarray(t, f) for t in (g_q, b_q, g_k, b_k, g_v, b_v, g_bn, b_bn))
    x = np.ascontiguousarray(x, f)
    xf = x.reshape(B, N, DIM)
    Xr = np.ascontiguousarray(xf.transpose(1, 0, 2).reshape(N * B, DIM))   # n-major rows
    xt = np.ascontiguousarray(Xr.T.reshape(8, 128, 128).transpose(1, 0, 2))  # [p, c, r]

    mn = np.zeros((128, 2), f)
    mn[0:64, 0] = 1.0
    mn[64:128, 1] = 1.0
    m2 = np.ascontiguousarray(mn.T)            # (2, 128)
    m2o = np.ascontiguousarray(mn[:, ::-1].T)  # opposite channel
    gb = np.stack([g_q, g_k, g_v, b_q, b_k, b_v, g_bn, b_bn], axis=1).astype(f)
    bo8 = (np.asarray(bo, f) / NC).astype(f)

    in_maps = []
    for i in range(NC):
        rows = slice(i * DPC, (i + 1) * DPC)
        head = i // 4
        wqk_c = np.concatenate([Wq[rows], Wk[rows]], axis=0).astype(f)       # (512, 1024)
        wqk = np.ascontiguousarray(wqk_c.T.reshape(8, 128, 512).transpose(1, 0, 2))
        wv_c = np.asarray(Wv[rows], f)                                        # (256, 1024)
        wv = np.ascontiguousarray(wv_c.T.reshape(8, 128, 256).transpose(1, 0, 2))
        WoC = np.asarray(Wo[:, rows], f)                                      # (1024, 256)
        wo = np.ascontiguousarray(WoC.T.reshape(2, 128, 1024).transpose(1, 0, 2))
        wos = np.ascontiguousarray(WoC.sum(1))                                # (1024,)
        hm = np.zeros((128, 2), f)
        hm[:, head] = 1.0
        in_maps.append({
            "xt": xt, "wqk": wqk, "wv": wv, "wo": wo,
            "wos": wos, "bo8": bo8, "hm": hm, "mn": mn, "m2": m2,
            "m2o": m2o, "gb": gb,
        })
    return in_maps


# --------------------------------------------------------------------------
# driver
# --------------------------------------------------------------------------

def _prep_inputs_v7(x, Wq, Wk, Wv, Wo, bo, g_q, b_q, g_k, b_k, g_v, b_v, g_bn, b_bn):
    """v2 prep + int16 fixed-point scales computed from the exact inputs."""
    f = np.float32
    in_maps = _prep_inputs_v2(x, Wq, Wk, Wv, Wo, bo, g_q, b_q, g_k, b_k,
                              g_v, b_v, g_bn, b_bn)
    xf = np.asarray(x, f).reshape(B, N, DIM)
    Xr = np.ascontiguousarray(xf.transpose(1, 0, 2).reshape(N * B, DIM))
    q = Xr @ np.asarray(Wq, f).T        # (128, 2048)
    k = Xr @ np.asarray(Wk, f).T
    v = Xr @ np.asarray(Wv, f).T
    ksw_rows = np.concatenate([np.arange(64, 128), np.arange(0, 64)])

    M = np.ones(16, f)                   # per-payload-column absmax over cores
    ysum_abs = np.zeros((128, DIM), f)
    Wof = np.asarray(Wo, f)
    for c in range(NC):
        sl = slice(c * DPC, (c + 1) * DPC)
        qc, kc, vc = q[:, sl], k[:, sl], v[:, sl]
        slot = 0 if c < 4 else 4
        M[slot + 0] = max(M[slot + 0], np.abs((qc * kc).sum(1)).max())
        M[slot + 1] = max(M[slot + 1], np.abs((qc * kc[ksw_rows]).sum(1)).max())
        M[slot + 2] = max(M[slot + 2], np.abs(qc.sum(1)).max())
        M[slot + 3] = max(M[slot + 3], np.abs(kc.sum(1)).max())
        M[8] = max(M[8], (qc ** 2).sum(1).max())
        M[9] = max(M[9], (kc ** 2).sum(1).max())
        vsq = vc ** 2
        ch_sq = np.stack([vsq[0:64].sum(0), vsq[64:128].sum(0)])     # (2, 256)
        ch_s = np.stack([vc[0:64].sum(0), vc[64:128].sum(0)])
        M[11] = max(M[11], np.abs(ch_sq[:, 0:128] + ch_sq[:, 128:256]).max())
        M[12] = M[11]
        M[13] = max(M[13], np.abs(ch_s[:, 0:128] + ch_s[:, 128:256]).max())
        M[14] = M[13]
        ysum_abs += np.abs(vc @ Wof[:, sl].T)
    sY = f(30000.0 / max(ysum_abs.max(), 1e-3))
    psc_row = (2.0 ** 22) / (M * 2.0)
    psc = np.broadcast_to(psc_row.astype(f), (128, 16)).copy()
    ipsc = np.broadcast_to((1.0 / psc_row).astype(f), (128, 16)).copy()
    isy = np.full((128, 1), 1.0 / sY, f)
    for i in range(NC):
        in_maps[i]["psc"] = psc
        in_maps[i]["ipsc"] = ipsc
        in_maps[i]["isy"] = isy
        in_maps[i]["hmy"] = (in_maps[i]["hm"] * sY).astype(f)
    return in_maps


def _prep_inputs(**inputs):
    if STRATEGY == "v1":
        return _prep_inputs_v1(**inputs)
    if STRATEGY in ("v7", "v8", "v9"):
        return _prep_inputs_v7(**inputs)
    return _prep_inputs_v2(**inputs)


def _postprocess(out128):
    return np.ascontiguousarray(
        out128.reshape(N, B, DIM).transpose(1, 0, 2).reshape(B, N, H, W)
    ).astype(np.float32)


def _get_program(reps=1):
    key = ("nc", STRATEGY, MM_DT, reps, NO_CC, CC_F32, CC_DIM, CC_UT)
    if key not in _PROG_CACHE:
        _PROG_CACHE[key] = _build_program(MM_DT, reps)
    return _PROG_CACHE[key]


def kernel(**inputs):
    from concourse.bass_utils import run_bass_kernel_spmd
    nc = _get_program()
    in_maps = _prep_inputs(**inputs)
    res = run_bass_kernel_spmd(nc, in_maps, list(range(NC)))
    return _postprocess(res.results[0]["out"])


def run_traced(inputs):
    """Like kernel() but with NTFF tracing; returns (output, BassKernelResults)."""
    from concourse.bass_utils import run_bass_kernel_spmd
    nc = _get_program()
    in_maps = _prep_inputs(**inputs)
    res = run_bass_kernel_spmd(nc, in_maps, list(range(NC)), trace=True)
    return _postprocess(res.results[0]["out"]), res


def run_sim(inputs):
    """Validate in the multi-core simulator; returns output."""
    from concourse.bass_interp import MultiCoreSim
    nc = _get_program()
    in_maps = _prep_inputs(**inputs)
    sim = MultiCoreSim(nc, num_cores=NC, trace=False)
    for i in range(NC):
        for k, v in in_maps[i].items():
            sim.cores[i].tensor(k)[:] = v
    sim.simulate()
    return _postprocess(np.array(sim.cores[0].tensor("out")))
